# revision 31
# baseline (speedup 1.0000x reference)
"""Grouped-GEMM (MoE expert FFN) kernel for 8 Trainium2 NeuronCores.

Problem: out[e, m, n] = sum_k x[e, m, k] * w[e, n, k] for m < m_sizes[e],
         zero elsewhere.  E=8, MAX_M=2048, K=2048, N=8192, fp32.

Default implementation: 1-level Winograd-Strassen (kernel_strassen,
~430us) -- 7/8 of the bf16 PE work of the direct kernel (kernel_base,
~445us, kept as BASS_IMPL=base fallback).

Shared structure
----------------------------------------------------------------------
* N-split sharding: every core computes ALL experts against its own
  (N/8)=1024-wide column slice of each expert's weights (perfect load
  balance; each weight element read once fleet-wide).
* exact-m: the moving free dim is the token count, so each expert does
  exactly m_e rows of PE work, zero padding (Strassen: ceil(m/2)).
* bf16 operands at 1 PE cycle/row; fp32 PSUM accumulate.

Winograd-Strassen specifics (see build_nc_strassen)
----------------------------------------------------------------------
* Per expert: M->2 x K->2x1024 x N->2x512 split, 7 products
  recombined with the all-ADD Winograd schedule (P4's sign folded into
  its weight combo), so every recombination op has <=1 PSUM operand.
* x-combos + w-combos computed on the DVE (vector) from raw quadrant
  streams -- HBM traffic stays at the direct kernel's 81MB/core.
  GpSimd must not do this work: it cannot read PSUM, costs ~2.3us/op,
  and blocks the out-DMA triggers queued behind it.
* Two nn' tiles share each PSUM bank ([128, 2*256] fp32) so 7 banks
  hold a product group and recombination ops are few and wide.
* bf16 recombination intermediates: the chip's activity monitor
  (HAM) power-throttles the PE to ~81% for ~160us when PE + DVE + DMA
  all run hot; bf16 intermediates cut DVE byte traffic enough to keep
  the throttle off (429960ns vs 445288ns fp32, rel err 1.26e-2 vs
  tolerance 2e-2).
* Queues: sync = raw w quadrants, scalar = raw x quadrants,
  gpsimd = output; first expert's tiles burst across all three.
* Expert order interleaves big/small so small (w-DMA-bound) experts'
  weight streams prefetch under the preceding big expert's PE time.
"""
import sys
import types

import ml_dtypes
import numpy as np

import concourse.bass as bass
import concourse.tile as tile
from concourse import bacc, mybir
from concourse.bass_utils import run_bass_kernel_spmd

P = 128          # partition dim / k-tile
N_CORES = 8
MB = 512         # max moving rows per matmul (one PSUM bank of fp32)

LAST_RESULT = None   # BassKernelResults of the most recent run (for tests)


def _install_profile_shim():
    """The agent image's antenv stub lacks axon_hooks; provide it so
    BASS_TRACE=1 profiling works instead of crashing."""
    if "antenv.axon_hooks" in sys.modules:
        return
    try:
        from trn_agent_boot.trn_boot import _ntff_profile_via_ctypes
        hook = _ntff_profile_via_ctypes("/opt/axon/libaxon_pjrt.so")
        mod = types.ModuleType("antenv.axon_hooks")
        mod.get_axon_ntff_profile_hook = lambda: hook
        sys.modules["antenv.axon_hooks"] = mod
        import antenv
        antenv.axon_hooks = mod
    except Exception:
        pass


def to_bf16(a: np.ndarray) -> np.ndarray:
    return np.asarray(a, dtype=ml_dtypes.bfloat16)


def blocks_of(m):
    """Even m-block sizes: ceil(m/MB) blocks, sizes differing by <=1."""
    nb = (m + MB - 1) // MB
    base = m // nb
    rem = m - base * nb
    return [base + (1 if i < rem else 0) for i in range(nb)]


def build_nc(m_list, K, NC_N, psum_bufs=8, w_bufs=20, x_bufs=6, out_bufs=3):
    """SPMD program for per-segment (expert) valid row counts m_list."""
    KK = K // P
    KH = KK // 2
    NN = NC_N // P
    SM = sum(m_list)

    nc = bacc.Bacc("TRN2", target_bir_lowering=False, debug=False,
                   num_devices=N_CORES)
    n_blocks = sum(len(blocks_of(m)) for m in m_list)
    # x^T, packed per (segment, m-block): [128, KK*mbs] used cols
    xsw = nc.dram_tensor("xsw", [n_blocks * P, KK * MB], mybir.dt.bfloat16,
                         kind="ExternalInput").ap()
    # weights, packed per (segment, nn): row ((seg*NN + nn)*P + p),
    # col kk*P + j  =  w[seg, nn*P + j, kk*P + p]
    wsw = nc.dram_tensor("wsw", [len(m_list) * NN * P, KK * P],
                         mybir.dt.bfloat16, kind="ExternalInput").ap()
    # output, transposed+interleaved: [p, nn*SM + m] = out[m, nn*P + p]
    out = nc.dram_tensor("out", [P, NN * SM], mybir.dt.bfloat16,
                         kind="ExternalOutput").ap()
    out3 = out.rearrange("p (a m) -> p a m", a=NN)

    with tile.TileContext(nc) as tc:
        with tc.tile_pool(name="wp", bufs=w_bufs) as wp, \
             tc.tile_pool(name="xp", bufs=x_bufs) as xp, \
             tc.tile_pool(name="op", bufs=out_bufs) as op, \
             tc.tile_pool(name="pp", bufs=psum_bufs, space="PSUM") as pp, \
             tc.tile_pool(name="wu", bufs=1) as wu:
            # PE warmup: dummy bf16 matmuls spanning the initial DMA wait
            # keep the HAM activity monitor engaged so the PE clock is at
            # 2.4 GHz when the first real tiles land (needs >=4us of
            # continuous matmul to fully ramp).
            wa_r = wu.tile([P, MB], mybir.dt.bfloat16, tag="war")
            nc.gpsimd.memset(wa_r[:], 0.0)
            wpss = [pp.tile([P, MB], mybir.dt.float32, tag="ps",
                            name="wps") for _ in range(4)]
            for i in range(8):
                nc.tensor.matmul(wpss[i % 4][:], wa_r[:, :P], wa_r[:],
                                 start=True, stop=True)
            blk = 0
            col0 = 0
            KQ = KK // 4
            n_segs = len(m_list)

            def load_x(mbs, blk, first):
                # x as 4 quarter-K tiles: the first matmul only waits
                # for a quarter of the block's x, not half.  The very
                # first block is split across all three DMA queues.
                xts = [xp.tile([P, KQ * MB], mybir.dt.bfloat16,
                               tag=f"x{q}", name=f"xt{q}")
                       for q in range(4)]
                engs = [nc.scalar, nc.sync, nc.gpsimd, nc.scalar] if first \
                    else [nc.scalar] * 4
                for q in range(4):
                    engs[q].dma_start(
                        out=xts[q][:, :KQ * mbs],
                        in_=xsw[blk * P:(blk + 1) * P,
                                q * KQ * mbs:(q + 1) * KQ * mbs])
                return xts

            for seg, m in enumerate(m_list):
                w_ts = []
                blocks = blocks_of(m)
                xts0 = None
                for nn in range(NN):
                    w_t = wp.tile([P, KK * P], mybir.dt.bfloat16, tag="w")
                    r0 = (seg * NN + nn) * P
                    if seg == 0 and nn == 0:
                        # critical first tile: split across all three DMA
                        # queues so the very first matmul starts ~5us in
                        engs4 = [nc.sync, nc.scalar, nc.gpsimd, nc.sync]
                        qc = (KK * P) // 4
                        for q in range(4):
                            engs4[q].dma_start(
                                out=w_t[:, q * qc:(q + 1) * qc],
                                in_=wsw[r0:r0 + P, q * qc:(q + 1) * qc])
                        # first block's x goes out right after the first
                        # weight tile, ahead of the remaining 3.5MB of
                        # first-expert weights
                        xts0 = load_x(blocks[0], blk, True)
                        blk += 1
                    else:
                        # first expert: odd nn tiles ride the (idle) gpsimd
                        # queue so the 4MB expert load halves in latency
                        eng = nc.gpsimd if (seg == 0 and nn % 2 == 1) \
                            else nc.sync
                        eng.dma_start(out=w_t[:], in_=wsw[r0:r0 + P, :])
                    w_ts.append(w_t)
                for bi, mbs in enumerate(blocks):
                    if seg == 0 and bi == 0:
                        xts = xts0
                    else:
                        xts = load_x(mbs, blk, False)
                        blk += 1
                    # last two segments: per-nn output DMAs spread across
                    # all queues overlap the final casts so the kernel
                    # tail is one small DMA, not one big consolidated one
                    tail_seg = seg >= n_segs - 2
                    o_t = op.tile([P, NN * MB], mybir.dt.bfloat16, tag="o")
                    tail_engs = [nc.gpsimd, nc.sync, nc.scalar, nc.gpsimd]
                    for nn in range(NN):
                        ps = pp.tile([P, MB], mybir.dt.float32, tag="ps",
                                     name="ps")
                        for kk in range(KK):
                            xt = xts[kk // KQ]
                            j = kk % KQ
                            nc.tensor.matmul(
                                ps[:, :mbs],
                                w_ts[nn][:, kk * P:(kk + 1) * P],
                                xt[:, j * mbs:(j + 1) * mbs],
                                start=(kk == 0), stop=(kk == KK - 1))
                        nc.vector.tensor_copy(
                            o_t[:, nn * mbs:(nn + 1) * mbs], ps[:, :mbs])
                        if tail_seg:
                            tail_engs[nn % 4].dma_start(
                                out=out3[:, nn, col0:col0 + mbs],
                                in_=o_t[:, nn * mbs:(nn + 1) * mbs])
                    if not tail_seg:
                        nc.gpsimd.dma_start(
                            out=out3[:, :, col0:col0 + mbs],
                            in_=o_t[:, :NN * mbs])
                    col0 += mbs
    nc.compile()
    return nc


_NC_CACHE = {}


def get_nc(m_list, K, NC_N, **kw):
    key = (tuple(m_list), K, NC_N, tuple(sorted(kw.items())))
    if key not in _NC_CACHE:
        _NC_CACHE[key] = build_nc(m_list, K, NC_N, **kw)
    return _NC_CACHE[key]


# ----------------------------------------------------------------------
# Strassen (1 level) variant: 7/8 of the PE work.
#
# Per expert (m rows, K=2048, per-core N slice 1024):
#   split M -> 2 halves of mh=ceil(m/2), K -> 2x1024, N -> 2x512.
#   7 products P_p = Ac_p @ Wc_p^T, each [mh, 1024] x [512, 1024]^T,
#   recombined into quadrants C11/C12/C21/C22 by vector+gpsimd.
#   x-combos (5 adds/subs per m-block) and w-combos (5 per expert-nn)
#   are computed on-device by the otherwise idle vector/gpsimd engines
#   so HBM traffic stays at the baseline 81MB/core.
# Queues: sync = raw w quadrants, scalar = raw x quadrants, gpsimd = out.
# PSUM: 7 banks per (nn', m-block) group + 1 spare for pipelining.
# ----------------------------------------------------------------------
MBS = 256        # Strassen m-block cap (SBUF-pressure bound)
KH = 8           # k-tiles per K-half
NH = 4           # n-tiles per N-half

# Winograd-Strassen operands.  Products (0-based banks):
#   ban0 = A11 B11t   ban1 = A12 B12t   ban2 = S4 B22t
#   ban3 = A22 T4't (= -P4_classic, sign folded)   ban4 = S1 T1t
#   ban5 = S2 T2t     ban6 = S3 T3t
# x-combos: S1 = A21+A22, S2 = S1-A11, S3 = A11-A21, S4 = A12-S2
# w-combos: T1 = B21-B11, T2 = B22-T1, T3 = B22-B21, T4' = B12-T2
# recombine (all adds): C11 = ban0+ban1 ; U2 = ban0+ban5 ; U3 = U2+ban6
#   U4 = U2+ban4 ; C12 = U4+ban2 ; C21 = U3+ban3 ; C22 = U3+ban4


def blocks_of_s(mh):
    nb = (mh + MBS - 1) // MBS
    base = mh // nb
    rem = mh - base * nb
    return [base + (1 if i < rem else 0) for i in range(nb)]


def build_nc_strassen(m_list, w_bufs=38, x_bufs=8, xc_bufs=6):
    """m_list: full per-expert row counts (processing order)."""
    mh_list = [(m + 1) // 2 for m in m_list]
    n_blocks = sum(len(blocks_of_s(mh)) for mh in mh_list)
    out_cols = 16 * sum(mh_list)

    nc = bacc.Bacc("TRN2", target_bir_lowering=False, debug=False,
                   num_devices=N_CORES)
    # raw x quadrants per (expert, block, quad): [128, KH*mbs] used cols
    xsw = nc.dram_tensor("xsw", [n_blocks * 4 * P, KH * MBS],
                         mybir.dt.bfloat16, kind="ExternalInput").ap()
    # raw w quadrants per (expert, nn', quad): [128, KH*128]
    wsw = nc.dram_tensor("wsw", [len(m_list) * NH * 4 * P, KH * P],
                         mybir.dt.bfloat16, kind="ExternalInput").ap()
    out = nc.dram_tensor("out", [P, out_cols], mybir.dt.bfloat16,
                         kind="ExternalOutput").ap()

    with tile.TileContext(nc) as tc:
        with tc.tile_pool(name="wrp", bufs=10) as wrp, \
             tc.tile_pool(name="wcp", bufs=w_bufs) as wcp, \
             tc.tile_pool(name="xrp", bufs=x_bufs) as xrp, \
             tc.tile_pool(name="xcp", bufs=xc_bufs) as xcp, \
             tc.tile_pool(name="opo", bufs=4) as opo, \
             tc.tile_pool(name="tpv", bufs=2) as tpv, \
             tc.tile_pool(name="ppp", bufs=8, space="PSUM") as ppp, \
             tc.tile_pool(name="wup", bufs=1) as wup:
            # PE warmup (HAM ramp) while first DMAs land
            wa_r = wup.tile([P, MB], mybir.dt.bfloat16, tag="war")
            nc.gpsimd.memset(wa_r[:], 0.0)
            wpss = [ppp.tile([P, MB], mybir.dt.float32, tag="ps",
                             name="wps") for _ in range(4)]
            for i in range(8):
                nc.tensor.matmul(wpss[i % 4][:], wa_r[:, :P], wa_r[:],
                                 start=True, stop=True)

            def load_w_nn(ei, nn, first):
                """DMA 4 raw quadrant tiles for one nn', build its 7
                stationary operands (4 combos on gpsimd + 3 raws).
                Winograd: T1=B21-B11 T2=B22-T1 T3=B22-B21 T4'=B12-T2."""
                raws = []
                for q in range(4):
                    r0 = ((ei * NH + nn) * 4 + q) * P
                    keep = q in (0, 1, 3)     # B11, B12, B22 stay
                    pool = wcp if keep else wrp
                    wr = pool.tile([P, KH * P], mybir.dt.bfloat16,
                                   tag="wc" if keep else "wt",
                                   name="wr")
                    if first and nn == 0:
                        engs = [nc.sync, nc.scalar, nc.gpsimd, nc.sync]
                        engs[q].dma_start(out=wr[:],
                                          in_=wsw[r0:r0 + P, :])
                    elif ei == 0:
                        # first expert: burst raw w across all 3 queues
                        # so pair-1 isn't starved at t~18us
                        engs3 = [nc.sync, nc.gpsimd, nc.scalar]
                        engs3[(nn * 4 + q) % 3].dma_start(
                            out=wr[:], in_=wsw[r0:r0 + P, :])
                    elif ei == 1:
                        eng = nc.gpsimd if (nn * 4 + q) % 2 else nc.sync
                        eng.dma_start(out=wr[:], in_=wsw[r0:r0 + P, :])
                    else:
                        nc.sync.dma_start(out=wr[:],
                                          in_=wsw[r0:r0 + P, :])
                    raws.append(wr)
                b11, b12, b21, b22 = raws
                cs = [wcp.tile([P, KH * P], mybir.dt.bfloat16, tag="wc",
                               name="wcc") for _ in range(4)]
                t1, t2, t3, t4 = cs
                # vector, not gpsimd: gpsimd is ~2.3us/op and would block
                # the out-DMA triggers queued behind it
                nc.vector.tensor_sub(t1[:], b21[:], b11[:])
                nc.vector.tensor_sub(t2[:], b22[:], t1[:])
                nc.vector.tensor_sub(t3[:], b22[:], b21[:])
                nc.vector.tensor_sub(t4[:], b12[:], t2[:])   # -T4
                # stationary operand of product p
                return [b11, b12, b22, t4, t1, t2, t3]

            blk = 0
            col0 = 0
            NPAIR = NH // 2
            for ei, m in enumerate(m_list):
                mh = mh_list[ei]
                if ei == 0:
                    # head: first nn' weights, then first x block, then
                    # the rest of the first expert's weights
                    wops = [load_w_nn(0, 0, True)]
                else:
                    wops = [load_w_nn(ei, nn, False) for nn in range(NH)]
                b0 = 0
                for bi, mbs in enumerate(blocks_of_s(mh)):
                    first_blk = ei == 0 and bi == 0
                    raws = []
                    for q in range(4):
                        xr = xrp.tile([P, KH * MBS], mybir.dt.bfloat16,
                                      tag="xr", name="xr")
                        if ei == 0 and bi <= 2:
                            # expert-0's x blocks otherwise serialize on
                            # the scalar queue (~50us for 4 blocks) while
                            # sync/gpsimd carry only 4.2MB of weights --
                            # that starved pair-0 of block 1 for ~9us and
                            # tripped a 10us K=4 HAM window.  Rotate the
                            # first three blocks across all three queues.
                            rot = [nc.scalar, nc.sync, nc.gpsimd]
                            engs = [rot[(bi + i) % 3] for i in range(4)]
                        else:
                            engs = [nc.scalar] * 4
                        engs[q].dma_start(
                            out=xr[:, :KH * mbs],
                            in_=xsw[(blk * 4 + q) * P:(blk * 4 + q + 1) * P,
                                    :KH * mbs])
                        raws.append(xr)
                    blk += 1
                    a11, a12, a21, a22 = raws
                    w = KH * mbs
                    cs = [xcp.tile([P, KH * MBS], mybir.dt.bfloat16,
                                   tag="xc", name="xcc") for _ in range(4)]
                    s1, s2, s3, s4 = cs
                    nc.vector.tensor_add(s1[:, :w], a21[:, :w], a22[:, :w])
                    nc.vector.tensor_sub(s2[:, :w], s1[:, :w], a11[:, :w])
                    nc.vector.tensor_sub(s3[:, :w], a11[:, :w], a21[:, :w])
                    nc.vector.tensor_sub(s4[:, :w], a12[:, :w], s2[:, :w])
                    xops = [a11, a12, s4, a22, s1, s2, s3]
                    if first_blk:
                        wops.extend(load_w_nn(0, nn, False)
                                    for nn in range(1, NH))
                    for npair in range(NPAIR):
                        # two nn' share each PSUM bank: [:, h*mbs:...]
                        # cold start: issue the very first pair's products
                        # in chain-ready order (raw-operand products first,
                        # then as each vector combo lands) so the PE never
                        # idles while the combo chains fill
                        if first_blk and npair == 0:
                            p_order = [0, 1, 3, 4, 5, 6, 2]
                        else:
                            p_order = list(range(7))
                        ban = [None] * 7
                        for p in p_order:
                            ps = ppp.tile([P, MB], mybir.dt.float32,
                                          tag="ps", name="ps")
                            for h in (0, 1):
                                wt = wops[npair * 2 + h][p]
                                for kk in range(KH):
                                    nc.tensor.matmul(
                                        ps[:, h * mbs:h * mbs + mbs],
                                        wt[:, kk * P:(kk + 1) * P],
                                        xops[p][:, kk * mbs:(kk + 1) * mbs],
                                        start=(h == 0 and kk == 0),
                                        stop=(h == 1 and kk == KH - 1))
                            ban[p] = ps
                        # all-add Winograd recombination, <=1 PSUM per op
                        o_t = opo.tile([P, 8 * MBS], mybir.dt.bfloat16,
                                       tag="o", name="o")
                        s = slice(0, 2 * mbs)
                        # bf16 intermediates: ~45% less DVE byte traffic,
                        # keeps the HAM power throttle off the PE
                        # (rel err 9.1e-3 vs 8.3e-3 fp32, budget 2e-2)
                        cp = tpv.tile([P, 2 * MBS], mybir.dt.bfloat16,
                                      tag="cp", name="cp")
                        u2 = tpv.tile([P, 2 * MBS], mybir.dt.bfloat16,
                                      tag="u2", name="u2")
                        u3 = tpv.tile([P, 2 * MBS], mybir.dt.bfloat16,
                                      tag="u3", name="u3")
                        u4 = tpv.tile([P, 2 * MBS], mybir.dt.bfloat16,
                                      tag="u4", name="u4")
                        CPY = mybir.ActivationFunctionType.Copy
                        nc.scalar.activation(cp[:, s], ban[0][:, s], CPY)
                        nc.vector.tensor_add(o_t[:, 0:2 * mbs],
                                             ban[1][:, s], cp[:, s])  # C11
                        nc.vector.tensor_add(u2[:, s], ban[5][:, s],
                                             cp[:, s])
                        nc.vector.tensor_add(u3[:, s], ban[6][:, s],
                                             u2[:, s])
                        nc.vector.tensor_add(u4[:, s], ban[4][:, s],
                                             u2[:, s])
                        nc.vector.tensor_add(o_t[:, 2 * mbs:4 * mbs],
                                             ban[2][:, s], u4[:, s])  # C12
                        nc.vector.tensor_add(o_t[:, 4 * mbs:6 * mbs],
                                             ban[3][:, s], u3[:, s])  # C21
                        nc.vector.tensor_add(o_t[:, 6 * mbs:8 * mbs],
                                             ban[4][:, s], u3[:, s])  # C22
                        if ei == len(m_list) - 1:
                            # tail: last expert's outputs ride all three
                            # queues in halves so the final DMA is small
                            h4 = 4 * mbs
                            t_engs = [nc.sync, nc.scalar]
                            for ti in range(2):
                                t_engs[ti].dma_start(
                                    out=out[:, col0 + ti * h4:
                                            col0 + (ti + 1) * h4],
                                    in_=o_t[:, ti * h4:(ti + 1) * h4])
                        else:
                            nc.gpsimd.dma_start(
                                out=out[:, col0:col0 + 8 * mbs],
                                in_=o_t[:, :8 * mbs])
                        col0 += 8 * mbs
                    b0 += mbs
    nc.compile()
    return nc


def pack_x_strassen(x_padded, order, m_all, K):
    parts = []
    for e in order:
        m = m_all[e]
        mh = (m + 1) // 2
        xp2 = np.zeros((2 * mh, K), dtype=np.float32)
        xp2[:m] = x_padded[e, :m, :]
        b0 = 0
        for mbs in blocks_of_s(mh):
            for r in (0, 1):
                for g in (0, 1):
                    q = xp2[r * mh + b0:r * mh + b0 + mbs,
                            g * (K // 2):(g + 1) * (K // 2)]
                    b = q.T.reshape(KH, P, mbs).transpose(1, 0, 2)
                    row = np.zeros((P, KH * MBS), dtype=np.float32)
                    row[:, :KH * mbs] = b.reshape(P, KH * mbs)
                    parts.append(row)
            b0 += mbs
    return to_bf16(np.concatenate(parts, axis=0))


def pack_w_strassen(stacked_weights, order, c, NC_N, K):
    parts = []
    for e in order:
        blkw = stacked_weights[e, c * NC_N:(c + 1) * NC_N, :]  # [1024, K]
        for nn in range(NH):
            for (h, g) in ((0, 0), (0, 1), (1, 0), (1, 1)):
                q = blkw[h * 512 + nn * P:h * 512 + (nn + 1) * P,
                         g * (K // 2):(g + 1) * (K // 2)]  # [128, 1024]
                # stationary: [p, kk*128 + j] = q[j, kk*128 + p]
                a = q.reshape(P, KH, P).transpose(2, 1, 0)
                parts.append(np.ascontiguousarray(a).reshape(P, KH * P))
    return to_bf16(np.concatenate(parts, axis=0))


def kernel_strassen(x_padded, stacked_weights, m_sizes):
    global LAST_RESULT
    x_padded = np.ascontiguousarray(np.asarray(x_padded, dtype=np.float32))
    stacked_weights = np.ascontiguousarray(
        np.asarray(stacked_weights, dtype=np.float32))
    E, MAX_M, K = x_padded.shape
    N = stacked_weights.shape[1]
    NC_N = N // N_CORES
    m_all = [int(min(max(int(mm), 0), MAX_M))
             for mm in np.asarray(m_sizes).astype(np.int64)]

    out_full = np.zeros((E, MAX_M, N), dtype=np.float32)
    order = [e for e in range(E) if m_all[e] > 0]
    if not order:
        return out_full
    order.sort(key=lambda e: -m_all[e])
    # interleave big/small so small experts' w prefetch hides under the
    # preceding big expert's long PE stretch (small experts are w-bound)
    desc = order
    inter = []
    i, j = 0, len(desc) - 1
    while i <= j:
        inter.append(desc[i])
        if j > i:
            inter.append(desc[j])
        i += 1
        j -= 1
    order = inter
    m_list = [m_all[e] for e in order]

    _install_profile_shim()
    key = ("strassen", tuple(m_list))
    if key not in _NC_CACHE:
        _NC_CACHE[key] = build_nc_strassen(m_list)
    nc = _NC_CACHE[key]

    xsw = pack_x_strassen(x_padded, order, m_all, K)
    in_maps = [{"xsw": xsw,
                "wsw": pack_w_strassen(stacked_weights, order, c, NC_N, K)}
               for c in range(N_CORES)]
    res = run_bass_kernel_spmd(nc, in_maps, list(range(N_CORES)))
    LAST_RESULT = res

    for c in range(N_CORES):
        o = np.asarray(res.results[c]["out"]).astype(np.float32)
        col = 0
        for i, e in enumerate(order):
            m = m_list[i]
            mh = (m + 1) // 2
            b0 = 0
            for mbs in blocks_of_s(mh):
                for npair in range(NH // 2):
                    for (r, h) in ((0, 0), (0, 1), (1, 0), (1, 1)):
                        for hh in (0, 1):
                            nn = npair * 2 + hh
                            t = o[:, col:col + mbs]  # [p, j]
                            r0 = r * mh + b0
                            r1 = min(r0 + mbs, m)
                            if r1 > r0:
                                n0 = c * NC_N + h * 512 + nn * P
                                out_full[e, r0:r1, n0:n0 + P] = \
                                    t[:, :r1 - r0].T
                            col += mbs
                b0 += mbs
    return out_full


def pack_x(x_padded, order, m_all, K):
    """x^T packed per (expert, m-block): [128, KK*mbs] rows, padded to
    the fixed [P, KK*MB] dram row width."""
    KK = K // P
    parts = []
    for e in order:
        m = m_all[e]
        mb0 = 0
        for mbs in blocks_of(m):
            b = x_padded[e, mb0:mb0 + mbs, :].T        # [K, mbs]
            b = b.reshape(KK, P, mbs).transpose(1, 0, 2)  # [P, KK, mbs]
            row = np.zeros((P, KK * MB), dtype=np.float32)
            row[:, :KK * mbs] = b.reshape(P, KK * mbs)
            parts.append(row)
            mb0 += mbs
    return to_bf16(np.concatenate(parts, axis=0))


def pack_w(stacked_weights, order, c, NC_N, K):
    """Weights per (expert, nn): [128, KK*P] with
    [p, kk*P+j] = w[e, c*NC_N + nn*P + j, kk*P + p]."""
    KK = K // P
    NN = NC_N // P
    parts = []
    for e in order:
        blk = stacked_weights[e, c * NC_N:(c + 1) * NC_N, :]  # [NC_N, K]
        a = blk.reshape(NN, P, KK, P).transpose(0, 3, 2, 1)   # [nn, p, kk, j]
        parts.append(np.ascontiguousarray(a).reshape(NN * P, KK * P))
    return to_bf16(np.concatenate(parts, axis=0))


def kernel(x_padded, stacked_weights, m_sizes):
    import os
    if os.environ.get("BASS_IMPL", "strassen") == "strassen":
        return kernel_strassen(x_padded, stacked_weights, m_sizes)
    return kernel_base(x_padded, stacked_weights, m_sizes)


def kernel_base(x_padded, stacked_weights, m_sizes):
    global LAST_RESULT
    x_padded = np.ascontiguousarray(np.asarray(x_padded, dtype=np.float32))
    stacked_weights = np.ascontiguousarray(
        np.asarray(stacked_weights, dtype=np.float32))
    E, MAX_M, K = x_padded.shape
    N = stacked_weights.shape[1]
    NC_N = N // N_CORES
    NN = NC_N // P
    m_all = [int(min(max(int(mm), 0), MAX_M))
             for mm in np.asarray(m_sizes).astype(np.int64)]

    out_full = np.zeros((E, MAX_M, N), dtype=np.float32)
    order = [e for e in range(E) if m_all[e] > 0]
    if not order:
        return out_full
    # descending size: the big first expert hides its own weight load,
    # and the small trailing experts ride 3-expert weight prefetch
    order.sort(key=lambda e: -m_all[e])
    m_list = [m_all[e] for e in order]
    SM = sum(m_list)

    _install_profile_shim()
    nc = get_nc(m_list, K, NC_N)

    xsw = pack_x(x_padded, order, m_all, K)
    in_maps = [{"xsw": xsw,
                "wsw": pack_w(stacked_weights, order, c, NC_N, K)}
               for c in range(N_CORES)]

    res = run_bass_kernel_spmd(nc, in_maps, list(range(N_CORES)))
    LAST_RESULT = res

    for c in range(N_CORES):
        o = np.asarray(res.results[c]["out"]).astype(np.float32)  # [P, NN*SM]
        outT = o.reshape(P, NN, SM).transpose(1, 0, 2).reshape(NC_N, SM)
        col = 0
        for i, e in enumerate(order):
            m = m_list[i]
            out_full[e, :m, c * NC_N:(c + 1) * NC_N] = outT[:, col:col + m].T
            col += m
    return out_full



# revision 33
# speedup vs baseline: 1.0036x; 1.0036x over previous
"""Grouped-GEMM (MoE expert FFN) kernel for 8 Trainium2 NeuronCores.

Problem: out[e, m, n] = sum_k x[e, m, k] * w[e, n, k] for m < m_sizes[e],
         zero elsewhere.  E=8, MAX_M=2048, K=2048, N=8192, fp32.

Default implementation: 1-level Winograd-Strassen (kernel_strassen,
~430us) -- 7/8 of the bf16 PE work of the direct kernel (kernel_base,
~445us, kept as BASS_IMPL=base fallback).

Shared structure
----------------------------------------------------------------------
* N-split sharding: every core computes ALL experts against its own
  (N/8)=1024-wide column slice of each expert's weights (perfect load
  balance; each weight element read once fleet-wide).
* exact-m: the moving free dim is the token count, so each expert does
  exactly m_e rows of PE work, zero padding (Strassen: ceil(m/2)).
* bf16 operands at 1 PE cycle/row; fp32 PSUM accumulate.

Winograd-Strassen specifics (see build_nc_strassen)
----------------------------------------------------------------------
* Per expert: M->2 x K->2x1024 x N->2x512 split, 7 products
  recombined with the all-ADD Winograd schedule (P4's sign folded into
  its weight combo), so every recombination op has <=1 PSUM operand.
* x-combos + w-combos computed on the DVE (vector) from raw quadrant
  streams -- HBM traffic stays at the direct kernel's 81MB/core.
  GpSimd must not do this work: it cannot read PSUM, costs ~2.3us/op,
  and blocks the out-DMA triggers queued behind it.
* Two nn' tiles share each PSUM bank ([128, 2*256] fp32) so 7 banks
  hold a product group and recombination ops are few and wide.
* bf16 recombination intermediates: the chip's activity monitor
  (HAM) power-throttles the PE to ~81% for ~160us when PE + DVE + DMA
  all run hot; bf16 intermediates cut DVE byte traffic enough to keep
  the throttle off (429960ns vs 445288ns fp32, rel err 1.26e-2 vs
  tolerance 2e-2).
* Queues: sync = raw w quadrants, scalar = raw x quadrants,
  gpsimd = output; first expert's tiles burst across all three.
* Expert order interleaves big/small so small (w-DMA-bound) experts'
  weight streams prefetch under the preceding big expert's PE time.
"""
import sys
import types

import ml_dtypes
import numpy as np

import concourse.bass as bass
import concourse.tile as tile
from concourse import bacc, mybir
from concourse.bass_utils import run_bass_kernel_spmd

P = 128          # partition dim / k-tile
N_CORES = 8
MB = 512         # max moving rows per matmul (one PSUM bank of fp32)

LAST_RESULT = None   # BassKernelResults of the most recent run (for tests)


def _install_profile_shim():
    """The agent image's antenv stub lacks axon_hooks; provide it so
    BASS_TRACE=1 profiling works instead of crashing."""
    if "antenv.axon_hooks" in sys.modules:
        return
    try:
        from trn_agent_boot.trn_boot import _ntff_profile_via_ctypes
        hook = _ntff_profile_via_ctypes("/opt/axon/libaxon_pjrt.so")
        mod = types.ModuleType("antenv.axon_hooks")
        mod.get_axon_ntff_profile_hook = lambda: hook
        sys.modules["antenv.axon_hooks"] = mod
        import antenv
        antenv.axon_hooks = mod
    except Exception:
        pass


def to_bf16(a: np.ndarray) -> np.ndarray:
    return np.asarray(a, dtype=ml_dtypes.bfloat16)


def blocks_of(m):
    """Even m-block sizes: ceil(m/MB) blocks, sizes differing by <=1."""
    nb = (m + MB - 1) // MB
    base = m // nb
    rem = m - base * nb
    return [base + (1 if i < rem else 0) for i in range(nb)]


def build_nc(m_list, K, NC_N, psum_bufs=8, w_bufs=20, x_bufs=6, out_bufs=3):
    """SPMD program for per-segment (expert) valid row counts m_list."""
    KK = K // P
    KH = KK // 2
    NN = NC_N // P
    SM = sum(m_list)

    nc = bacc.Bacc("TRN2", target_bir_lowering=False, debug=False,
                   num_devices=N_CORES)
    n_blocks = sum(len(blocks_of(m)) for m in m_list)
    # x^T, packed per (segment, m-block): [128, KK*mbs] used cols
    xsw = nc.dram_tensor("xsw", [n_blocks * P, KK * MB], mybir.dt.bfloat16,
                         kind="ExternalInput").ap()
    # weights, packed per (segment, nn): row ((seg*NN + nn)*P + p),
    # col kk*P + j  =  w[seg, nn*P + j, kk*P + p]
    wsw = nc.dram_tensor("wsw", [len(m_list) * NN * P, KK * P],
                         mybir.dt.bfloat16, kind="ExternalInput").ap()
    # output, transposed+interleaved: [p, nn*SM + m] = out[m, nn*P + p]
    out = nc.dram_tensor("out", [P, NN * SM], mybir.dt.bfloat16,
                         kind="ExternalOutput").ap()
    out3 = out.rearrange("p (a m) -> p a m", a=NN)

    with tile.TileContext(nc) as tc:
        with tc.tile_pool(name="wp", bufs=w_bufs) as wp, \
             tc.tile_pool(name="xp", bufs=x_bufs) as xp, \
             tc.tile_pool(name="op", bufs=out_bufs) as op, \
             tc.tile_pool(name="pp", bufs=psum_bufs, space="PSUM") as pp, \
             tc.tile_pool(name="wu", bufs=1) as wu:
            # PE warmup: dummy bf16 matmuls spanning the initial DMA wait
            # keep the HAM activity monitor engaged so the PE clock is at
            # 2.4 GHz when the first real tiles land (needs >=4us of
            # continuous matmul to fully ramp).
            wa_r = wu.tile([P, MB], mybir.dt.bfloat16, tag="war")
            nc.gpsimd.memset(wa_r[:], 0.0)
            wpss = [pp.tile([P, MB], mybir.dt.float32, tag="ps",
                            name="wps") for _ in range(4)]
            for i in range(8):
                nc.tensor.matmul(wpss[i % 4][:], wa_r[:, :P], wa_r[:],
                                 start=True, stop=True)
            blk = 0
            col0 = 0
            KQ = KK // 4
            n_segs = len(m_list)

            def load_x(mbs, blk, first):
                # x as 4 quarter-K tiles: the first matmul only waits
                # for a quarter of the block's x, not half.  The very
                # first block is split across all three DMA queues.
                xts = [xp.tile([P, KQ * MB], mybir.dt.bfloat16,
                               tag=f"x{q}", name=f"xt{q}")
                       for q in range(4)]
                engs = [nc.scalar, nc.sync, nc.gpsimd, nc.scalar] if first \
                    else [nc.scalar] * 4
                for q in range(4):
                    engs[q].dma_start(
                        out=xts[q][:, :KQ * mbs],
                        in_=xsw[blk * P:(blk + 1) * P,
                                q * KQ * mbs:(q + 1) * KQ * mbs])
                return xts

            for seg, m in enumerate(m_list):
                w_ts = []
                blocks = blocks_of(m)
                xts0 = None
                for nn in range(NN):
                    w_t = wp.tile([P, KK * P], mybir.dt.bfloat16, tag="w")
                    r0 = (seg * NN + nn) * P
                    if seg == 0 and nn == 0:
                        # critical first tile: split across all three DMA
                        # queues so the very first matmul starts ~5us in
                        engs4 = [nc.sync, nc.scalar, nc.gpsimd, nc.sync]
                        qc = (KK * P) // 4
                        for q in range(4):
                            engs4[q].dma_start(
                                out=w_t[:, q * qc:(q + 1) * qc],
                                in_=wsw[r0:r0 + P, q * qc:(q + 1) * qc])
                        # first block's x goes out right after the first
                        # weight tile, ahead of the remaining 3.5MB of
                        # first-expert weights
                        xts0 = load_x(blocks[0], blk, True)
                        blk += 1
                    else:
                        # first expert: odd nn tiles ride the (idle) gpsimd
                        # queue so the 4MB expert load halves in latency
                        eng = nc.gpsimd if (seg == 0 and nn % 2 == 1) \
                            else nc.sync
                        eng.dma_start(out=w_t[:], in_=wsw[r0:r0 + P, :])
                    w_ts.append(w_t)
                for bi, mbs in enumerate(blocks):
                    if seg == 0 and bi == 0:
                        xts = xts0
                    else:
                        xts = load_x(mbs, blk, False)
                        blk += 1
                    # last two segments: per-nn output DMAs spread across
                    # all queues overlap the final casts so the kernel
                    # tail is one small DMA, not one big consolidated one
                    tail_seg = seg >= n_segs - 2
                    o_t = op.tile([P, NN * MB], mybir.dt.bfloat16, tag="o")
                    tail_engs = [nc.gpsimd, nc.sync, nc.scalar, nc.gpsimd]
                    for nn in range(NN):
                        ps = pp.tile([P, MB], mybir.dt.float32, tag="ps",
                                     name="ps")
                        for kk in range(KK):
                            xt = xts[kk // KQ]
                            j = kk % KQ
                            nc.tensor.matmul(
                                ps[:, :mbs],
                                w_ts[nn][:, kk * P:(kk + 1) * P],
                                xt[:, j * mbs:(j + 1) * mbs],
                                start=(kk == 0), stop=(kk == KK - 1))
                        nc.vector.tensor_copy(
                            o_t[:, nn * mbs:(nn + 1) * mbs], ps[:, :mbs])
                        if tail_seg:
                            tail_engs[nn % 4].dma_start(
                                out=out3[:, nn, col0:col0 + mbs],
                                in_=o_t[:, nn * mbs:(nn + 1) * mbs])
                    if not tail_seg:
                        nc.gpsimd.dma_start(
                            out=out3[:, :, col0:col0 + mbs],
                            in_=o_t[:, :NN * mbs])
                    col0 += mbs
    nc.compile()
    return nc


_NC_CACHE = {}


def get_nc(m_list, K, NC_N, **kw):
    key = (tuple(m_list), K, NC_N, tuple(sorted(kw.items())))
    if key not in _NC_CACHE:
        _NC_CACHE[key] = build_nc(m_list, K, NC_N, **kw)
    return _NC_CACHE[key]


# ----------------------------------------------------------------------
# Strassen (1 level) variant: 7/8 of the PE work.
#
# Per expert (m rows, K=2048, per-core N slice 1024):
#   split M -> 2 halves of mh=ceil(m/2), K -> 2x1024, N -> 2x512.
#   7 products P_p = Ac_p @ Wc_p^T, each [mh, 1024] x [512, 1024]^T,
#   recombined into quadrants C11/C12/C21/C22 by vector+gpsimd.
#   x-combos (5 adds/subs per m-block) and w-combos (5 per expert-nn)
#   are computed on-device by the otherwise idle vector/gpsimd engines
#   so HBM traffic stays at the baseline 81MB/core.
# Queues: sync = raw w quadrants, scalar = raw x quadrants, gpsimd = out.
# PSUM: 7 banks per (nn', m-block) group + 1 spare for pipelining.
# ----------------------------------------------------------------------
MBS = 256        # Strassen m-block cap (SBUF-pressure bound)
KH = 8           # k-tiles per K-half
NH = 4           # n-tiles per N-half

# Winograd-Strassen operands.  Products (0-based banks):
#   ban0 = A11 B11t   ban1 = A12 B12t   ban2 = S4 B22t
#   ban3 = A22 T4't (= -P4_classic, sign folded)   ban4 = S1 T1t
#   ban5 = S2 T2t     ban6 = S3 T3t
# x-combos: S1 = A21+A22, S2 = S1-A11, S3 = A11-A21, S4 = A12-S2
# w-combos: T1 = B21-B11, T2 = B22-T1, T3 = B22-B21, T4' = B12-T2
# recombine (all adds): C11 = ban0+ban1 ; U2 = ban0+ban5 ; U3 = U2+ban6
#   U4 = U2+ban4 ; C12 = U4+ban2 ; C21 = U3+ban3 ; C22 = U3+ban4


def blocks_of_s(mh):
    nb = (mh + MBS - 1) // MBS
    base = mh // nb
    rem = mh - base * nb
    return [base + (1 if i < rem else 0) for i in range(nb)]


def build_nc_strassen(m_list, w_bufs=38, x_bufs=8, xc_bufs=6):
    """m_list: full per-expert row counts (processing order)."""
    mh_list = [(m + 1) // 2 for m in m_list]
    n_blocks = sum(len(blocks_of_s(mh)) for mh in mh_list)
    out_cols = 16 * sum(mh_list)

    nc = bacc.Bacc("TRN2", target_bir_lowering=False, debug=False,
                   num_devices=N_CORES)
    # raw x quadrants per (expert, block, quad): [128, KH*mbs] used cols
    xsw = nc.dram_tensor("xsw", [n_blocks * 4 * P, KH * MBS],
                         mybir.dt.bfloat16, kind="ExternalInput").ap()
    # raw w quadrants per (expert, nn', quad): [128, KH*128]
    wsw = nc.dram_tensor("wsw", [len(m_list) * NH * 4 * P, KH * P],
                         mybir.dt.bfloat16, kind="ExternalInput").ap()
    out = nc.dram_tensor("out", [P, out_cols], mybir.dt.bfloat16,
                         kind="ExternalOutput").ap()

    with tile.TileContext(nc) as tc:
        with tc.tile_pool(name="wrp", bufs=10) as wrp, \
             tc.tile_pool(name="wcp", bufs=w_bufs) as wcp, \
             tc.tile_pool(name="xrp", bufs=x_bufs) as xrp, \
             tc.tile_pool(name="xcp", bufs=xc_bufs) as xcp, \
             tc.tile_pool(name="opo", bufs=4) as opo, \
             tc.tile_pool(name="tpv", bufs=2) as tpv, \
             tc.tile_pool(name="ppp", bufs=8, space="PSUM") as ppp, \
             tc.tile_pool(name="wup", bufs=1) as wup:
            # PE warmup (HAM ramp) while first DMAs land
            wa_r = wup.tile([P, MB], mybir.dt.bfloat16, tag="war")
            nc.gpsimd.memset(wa_r[:], 0.0)
            wpss = [ppp.tile([P, MB], mybir.dt.float32, tag="ps",
                             name="wps") for _ in range(4)]
            for i in range(8):
                nc.tensor.matmul(wpss[i % 4][:], wa_r[:, :P], wa_r[:],
                                 start=True, stop=True)

            def load_w_nn(ei, nn, first):
                """DMA 4 raw quadrant tiles for one nn', build its 7
                stationary operands (4 combos on gpsimd + 3 raws).
                Winograd: T1=B21-B11 T2=B22-T1 T3=B22-B21 T4'=B12-T2."""
                raws = []
                for q in range(4):
                    r0 = ((ei * NH + nn) * 4 + q) * P
                    keep = q in (0, 1, 3)     # B11, B12, B22 stay
                    pool = wcp if keep else wrp
                    wr = pool.tile([P, KH * P], mybir.dt.bfloat16,
                                   tag="wc" if keep else "wt",
                                   name="wr")
                    if first and nn == 0:
                        engs = [nc.sync, nc.scalar, nc.gpsimd, nc.sync]
                        engs[q].dma_start(out=wr[:],
                                          in_=wsw[r0:r0 + P, :])
                    elif ei == 0:
                        # first expert: burst raw w across all 3 queues
                        # so pair-1 isn't starved at t~18us
                        engs3 = [nc.sync, nc.gpsimd, nc.scalar]
                        engs3[(nn * 4 + q) % 3].dma_start(
                            out=wr[:], in_=wsw[r0:r0 + P, :])
                    elif ei == 1:
                        eng = nc.gpsimd if (nn * 4 + q) % 2 else nc.sync
                        eng.dma_start(out=wr[:], in_=wsw[r0:r0 + P, :])
                    else:
                        nc.sync.dma_start(out=wr[:],
                                          in_=wsw[r0:r0 + P, :])
                    raws.append(wr)
                b11, b12, b21, b22 = raws
                cs = [wcp.tile([P, KH * P], mybir.dt.bfloat16, tag="wc",
                               name="wcc") for _ in range(4)]
                t1, t2, t3, t4 = cs
                # vector, not gpsimd: gpsimd is ~2.3us/op and would block
                # the out-DMA triggers queued behind it
                nc.vector.tensor_sub(t1[:], b21[:], b11[:])
                nc.vector.tensor_sub(t2[:], b22[:], t1[:])
                nc.vector.tensor_sub(t3[:], b22[:], b21[:])
                nc.vector.tensor_sub(t4[:], b12[:], t2[:])   # -T4
                # stationary operand of product p
                return [b11, b12, b22, t4, t1, t2, t3]

            blk = 0
            col0 = 0
            NPAIR = NH // 2
            for ei, m in enumerate(m_list):
                mh = mh_list[ei]
                if ei == 0:
                    # head: first nn' weights, then first x block, then
                    # the rest of the first expert's weights
                    wops = [load_w_nn(0, 0, True)]
                else:
                    wops = [load_w_nn(ei, nn, False) for nn in range(NH)]
                b0 = 0
                for bi, mbs in enumerate(blocks_of_s(mh)):
                    first_blk = ei == 0 and bi == 0
                    raws = []
                    for q in range(4):
                        xr = xrp.tile([P, KH * MBS], mybir.dt.bfloat16,
                                      tag="xr", name="xr")
                        if ei == 0 and bi <= 2:
                            # expert-0's x blocks otherwise serialize on
                            # the scalar queue (~50us for 4 blocks) while
                            # sync/gpsimd carry only 4.2MB of weights --
                            # that starved pair-0 of block 1 for ~9us and
                            # tripped a 10us K=4 HAM window.  Rotate the
                            # first three blocks across all three queues.
                            rot = [nc.scalar, nc.sync, nc.gpsimd]
                            engs = [rot[(bi + i) % 3] for i in range(4)]
                        else:
                            engs = [nc.scalar] * 4
                        engs[q].dma_start(
                            out=xr[:, :KH * mbs],
                            in_=xsw[(blk * 4 + q) * P:(blk * 4 + q + 1) * P,
                                    :KH * mbs])
                        raws.append(xr)
                    blk += 1
                    a11, a12, a21, a22 = raws
                    w = KH * mbs
                    cs = [xcp.tile([P, KH * MBS], mybir.dt.bfloat16,
                                   tag="xc", name="xcc") for _ in range(4)]
                    s1, s2, s3, s4 = cs
                    nc.vector.tensor_add(s1[:, :w], a21[:, :w], a22[:, :w])
                    nc.vector.tensor_sub(s2[:, :w], s1[:, :w], a11[:, :w])
                    nc.vector.tensor_sub(s3[:, :w], a11[:, :w], a21[:, :w])
                    nc.vector.tensor_sub(s4[:, :w], a12[:, :w], s2[:, :w])
                    xops = [a11, a12, s4, a22, s1, s2, s3]
                    if first_blk:
                        # nn1 now; nn2/nn3 are deferred until after
                        # pair-0's recombination is EMITTED: the vector
                        # engine is in-order, and pair-1's 2nd product
                        # waits on pair-0's first C-op (bank free) --
                        # which otherwise sits behind 8 combo ops in the
                        # vector queue (8.6us PE gap at t~20us).
                        wops.append(load_w_nn(0, 1, False))
                    for npair in range(NPAIR):
                        if first_blk and npair == 1:
                            wops.extend(load_w_nn(0, nn, False)
                                        for nn in (2, 3))
                        # two nn' share each PSUM bank: [:, h*mbs:...]
                        ban = []
                        for p in range(7):
                            ps = ppp.tile([P, MB], mybir.dt.float32,
                                          tag="ps", name="ps")
                            for h in (0, 1):
                                wt = wops[npair * 2 + h][p]
                                for kk in range(KH):
                                    nc.tensor.matmul(
                                        ps[:, h * mbs:h * mbs + mbs],
                                        wt[:, kk * P:(kk + 1) * P],
                                        xops[p][:, kk * mbs:(kk + 1) * mbs],
                                        start=(h == 0 and kk == 0),
                                        stop=(h == 1 and kk == KH - 1))
                            ban.append(ps)
                        # all-add Winograd recombination, <=1 PSUM per op
                        o_t = opo.tile([P, 8 * MBS], mybir.dt.bfloat16,
                                       tag="o", name="o")
                        s = slice(0, 2 * mbs)
                        # bf16 intermediates: ~45% less DVE byte traffic,
                        # keeps the HAM power throttle off the PE
                        # (rel err 9.1e-3 vs 8.3e-3 fp32, budget 2e-2)
                        cp = tpv.tile([P, 2 * MBS], mybir.dt.bfloat16,
                                      tag="cp", name="cp")
                        u2 = tpv.tile([P, 2 * MBS], mybir.dt.bfloat16,
                                      tag="u2", name="u2")
                        u3 = tpv.tile([P, 2 * MBS], mybir.dt.bfloat16,
                                      tag="u3", name="u3")
                        u4 = tpv.tile([P, 2 * MBS], mybir.dt.bfloat16,
                                      tag="u4", name="u4")
                        CPY = mybir.ActivationFunctionType.Copy
                        nc.scalar.activation(cp[:, s], ban[0][:, s], CPY)
                        nc.vector.tensor_add(o_t[:, 0:2 * mbs],
                                             ban[1][:, s], cp[:, s])  # C11
                        nc.vector.tensor_add(u2[:, s], ban[5][:, s],
                                             cp[:, s])
                        nc.vector.tensor_add(u3[:, s], ban[6][:, s],
                                             u2[:, s])
                        nc.vector.tensor_add(u4[:, s], ban[4][:, s],
                                             u2[:, s])
                        nc.vector.tensor_add(o_t[:, 2 * mbs:4 * mbs],
                                             ban[2][:, s], u4[:, s])  # C12
                        nc.vector.tensor_add(o_t[:, 4 * mbs:6 * mbs],
                                             ban[3][:, s], u3[:, s])  # C21
                        nc.vector.tensor_add(o_t[:, 6 * mbs:8 * mbs],
                                             ban[4][:, s], u3[:, s])  # C22
                        if ei == len(m_list) - 1:
                            # tail: last expert's outputs ride all three
                            # queues in halves so the final DMA is small
                            h4 = 4 * mbs
                            t_engs = [nc.sync, nc.scalar]
                            for ti in range(2):
                                t_engs[ti].dma_start(
                                    out=out[:, col0 + ti * h4:
                                            col0 + (ti + 1) * h4],
                                    in_=o_t[:, ti * h4:(ti + 1) * h4])
                        else:
                            nc.gpsimd.dma_start(
                                out=out[:, col0:col0 + 8 * mbs],
                                in_=o_t[:, :8 * mbs])
                        col0 += 8 * mbs
                    b0 += mbs
    nc.compile()
    return nc


def pack_x_strassen(x_padded, order, m_all, K):
    parts = []
    for e in order:
        m = m_all[e]
        mh = (m + 1) // 2
        xp2 = np.zeros((2 * mh, K), dtype=np.float32)
        xp2[:m] = x_padded[e, :m, :]
        b0 = 0
        for mbs in blocks_of_s(mh):
            for r in (0, 1):
                for g in (0, 1):
                    q = xp2[r * mh + b0:r * mh + b0 + mbs,
                            g * (K // 2):(g + 1) * (K // 2)]
                    b = q.T.reshape(KH, P, mbs).transpose(1, 0, 2)
                    row = np.zeros((P, KH * MBS), dtype=np.float32)
                    row[:, :KH * mbs] = b.reshape(P, KH * mbs)
                    parts.append(row)
            b0 += mbs
    return to_bf16(np.concatenate(parts, axis=0))


def pack_w_strassen(stacked_weights, order, c, NC_N, K):
    parts = []
    for e in order:
        blkw = stacked_weights[e, c * NC_N:(c + 1) * NC_N, :]  # [1024, K]
        for nn in range(NH):
            for (h, g) in ((0, 0), (0, 1), (1, 0), (1, 1)):
                q = blkw[h * 512 + nn * P:h * 512 + (nn + 1) * P,
                         g * (K // 2):(g + 1) * (K // 2)]  # [128, 1024]
                # stationary: [p, kk*128 + j] = q[j, kk*128 + p]
                a = q.reshape(P, KH, P).transpose(2, 1, 0)
                parts.append(np.ascontiguousarray(a).reshape(P, KH * P))
    return to_bf16(np.concatenate(parts, axis=0))


def kernel_strassen(x_padded, stacked_weights, m_sizes):
    global LAST_RESULT
    x_padded = np.ascontiguousarray(np.asarray(x_padded, dtype=np.float32))
    stacked_weights = np.ascontiguousarray(
        np.asarray(stacked_weights, dtype=np.float32))
    E, MAX_M, K = x_padded.shape
    N = stacked_weights.shape[1]
    NC_N = N // N_CORES
    m_all = [int(min(max(int(mm), 0), MAX_M))
             for mm in np.asarray(m_sizes).astype(np.int64)]

    out_full = np.zeros((E, MAX_M, N), dtype=np.float32)
    order = [e for e in range(E) if m_all[e] > 0]
    if not order:
        return out_full
    order.sort(key=lambda e: -m_all[e])
    # interleave big/small so small experts' w prefetch hides under the
    # preceding big expert's long PE stretch (small experts are w-bound)
    desc = order
    inter = []
    i, j = 0, len(desc) - 1
    while i <= j:
        inter.append(desc[i])
        if j > i:
            inter.append(desc[j])
        i += 1
        j -= 1
    order = inter
    m_list = [m_all[e] for e in order]

    _install_profile_shim()
    key = ("strassen", tuple(m_list))
    if key not in _NC_CACHE:
        _NC_CACHE[key] = build_nc_strassen(m_list)
    nc = _NC_CACHE[key]

    xsw = pack_x_strassen(x_padded, order, m_all, K)
    in_maps = [{"xsw": xsw,
                "wsw": pack_w_strassen(stacked_weights, order, c, NC_N, K)}
               for c in range(N_CORES)]
    res = run_bass_kernel_spmd(nc, in_maps, list(range(N_CORES)))
    LAST_RESULT = res

    for c in range(N_CORES):
        o = np.asarray(res.results[c]["out"]).astype(np.float32)
        col = 0
        for i, e in enumerate(order):
            m = m_list[i]
            mh = (m + 1) // 2
            b0 = 0
            for mbs in blocks_of_s(mh):
                for npair in range(NH // 2):
                    for (r, h) in ((0, 0), (0, 1), (1, 0), (1, 1)):
                        for hh in (0, 1):
                            nn = npair * 2 + hh
                            t = o[:, col:col + mbs]  # [p, j]
                            r0 = r * mh + b0
                            r1 = min(r0 + mbs, m)
                            if r1 > r0:
                                n0 = c * NC_N + h * 512 + nn * P
                                out_full[e, r0:r1, n0:n0 + P] = \
                                    t[:, :r1 - r0].T
                            col += mbs
                b0 += mbs
    return out_full


def pack_x(x_padded, order, m_all, K):
    """x^T packed per (expert, m-block): [128, KK*mbs] rows, padded to
    the fixed [P, KK*MB] dram row width."""
    KK = K // P
    parts = []
    for e in order:
        m = m_all[e]
        mb0 = 0
        for mbs in blocks_of(m):
            b = x_padded[e, mb0:mb0 + mbs, :].T        # [K, mbs]
            b = b.reshape(KK, P, mbs).transpose(1, 0, 2)  # [P, KK, mbs]
            row = np.zeros((P, KK * MB), dtype=np.float32)
            row[:, :KK * mbs] = b.reshape(P, KK * mbs)
            parts.append(row)
            mb0 += mbs
    return to_bf16(np.concatenate(parts, axis=0))


def pack_w(stacked_weights, order, c, NC_N, K):
    """Weights per (expert, nn): [128, KK*P] with
    [p, kk*P+j] = w[e, c*NC_N + nn*P + j, kk*P + p]."""
    KK = K // P
    NN = NC_N // P
    parts = []
    for e in order:
        blk = stacked_weights[e, c * NC_N:(c + 1) * NC_N, :]  # [NC_N, K]
        a = blk.reshape(NN, P, KK, P).transpose(0, 3, 2, 1)   # [nn, p, kk, j]
        parts.append(np.ascontiguousarray(a).reshape(NN * P, KK * P))
    return to_bf16(np.concatenate(parts, axis=0))


def kernel(x_padded, stacked_weights, m_sizes):
    import os
    if os.environ.get("BASS_IMPL", "strassen") == "strassen":
        return kernel_strassen(x_padded, stacked_weights, m_sizes)
    return kernel_base(x_padded, stacked_weights, m_sizes)


def kernel_base(x_padded, stacked_weights, m_sizes):
    global LAST_RESULT
    x_padded = np.ascontiguousarray(np.asarray(x_padded, dtype=np.float32))
    stacked_weights = np.ascontiguousarray(
        np.asarray(stacked_weights, dtype=np.float32))
    E, MAX_M, K = x_padded.shape
    N = stacked_weights.shape[1]
    NC_N = N // N_CORES
    NN = NC_N // P
    m_all = [int(min(max(int(mm), 0), MAX_M))
             for mm in np.asarray(m_sizes).astype(np.int64)]

    out_full = np.zeros((E, MAX_M, N), dtype=np.float32)
    order = [e for e in range(E) if m_all[e] > 0]
    if not order:
        return out_full
    # descending size: the big first expert hides its own weight load,
    # and the small trailing experts ride 3-expert weight prefetch
    order.sort(key=lambda e: -m_all[e])
    m_list = [m_all[e] for e in order]
    SM = sum(m_list)

    _install_profile_shim()
    nc = get_nc(m_list, K, NC_N)

    xsw = pack_x(x_padded, order, m_all, K)
    in_maps = [{"xsw": xsw,
                "wsw": pack_w(stacked_weights, order, c, NC_N, K)}
               for c in range(N_CORES)]

    res = run_bass_kernel_spmd(nc, in_maps, list(range(N_CORES)))
    LAST_RESULT = res

    for c in range(N_CORES):
        o = np.asarray(res.results[c]["out"]).astype(np.float32)  # [P, NN*SM]
        outT = o.reshape(P, NN, SM).transpose(1, 0, 2).reshape(NC_N, SM)
        col = 0
        for i, e in enumerate(order):
            m = m_list[i]
            out_full[e, :m, c * NC_N:(c + 1) * NC_N] = outT[:, col:col + m].T
            col += m
    return out_full



# revision 35
# speedup vs baseline: 1.0123x; 1.0086x over previous
"""Grouped-GEMM (MoE expert FFN) kernel for 8 Trainium2 NeuronCores.

Problem: out[e, m, n] = sum_k x[e, m, k] * w[e, n, k] for m < m_sizes[e],
         zero elsewhere.  E=8, MAX_M=2048, K=2048, N=8192, fp32.

Default implementation: 1-level Winograd-Strassen (kernel_strassen,
~430us) -- 7/8 of the bf16 PE work of the direct kernel (kernel_base,
~445us, kept as BASS_IMPL=base fallback).

Shared structure
----------------------------------------------------------------------
* N-split sharding: every core computes ALL experts against its own
  (N/8)=1024-wide column slice of each expert's weights (perfect load
  balance; each weight element read once fleet-wide).
* exact-m: the moving free dim is the token count, so each expert does
  exactly m_e rows of PE work, zero padding (Strassen: ceil(m/2)).
* bf16 operands at 1 PE cycle/row; fp32 PSUM accumulate.

Winograd-Strassen specifics (see build_nc_strassen)
----------------------------------------------------------------------
* Per expert: M->2 x K->2x1024 x N->2x512 split, 7 products
  recombined with the all-ADD Winograd schedule (P4's sign folded into
  its weight combo), so every recombination op has <=1 PSUM operand.
* x-combos + w-combos computed on the DVE (vector) from raw quadrant
  streams -- HBM traffic stays at the direct kernel's 81MB/core.
  GpSimd must not do this work: it cannot read PSUM, costs ~2.3us/op,
  and blocks the out-DMA triggers queued behind it.
* Two nn' tiles share each PSUM bank ([128, 2*256] fp32) so 7 banks
  hold a product group and recombination ops are few and wide.
* bf16 recombination intermediates: the chip's activity monitor
  (HAM) power-throttles the PE to ~81% for ~160us when PE + DVE + DMA
  all run hot; bf16 intermediates cut DVE byte traffic enough to keep
  the throttle off (429960ns vs 445288ns fp32, rel err 1.26e-2 vs
  tolerance 2e-2).
* Queues: sync = raw w quadrants, scalar = raw x quadrants,
  gpsimd = output; first expert's tiles burst across all three.
* Expert order interleaves big/small so small (w-DMA-bound) experts'
  weight streams prefetch under the preceding big expert's PE time.
"""
import sys
import types

import ml_dtypes
import numpy as np

import concourse.bass as bass
import concourse.tile as tile
from concourse import bacc, mybir
from concourse.bass_utils import run_bass_kernel_spmd

P = 128          # partition dim / k-tile
N_CORES = 8
MB = 512         # max moving rows per matmul (one PSUM bank of fp32)

LAST_RESULT = None   # BassKernelResults of the most recent run (for tests)


def _install_profile_shim():
    """The agent image's antenv stub lacks axon_hooks; provide it so
    BASS_TRACE=1 profiling works instead of crashing."""
    if "antenv.axon_hooks" in sys.modules:
        return
    try:
        from trn_agent_boot.trn_boot import _ntff_profile_via_ctypes
        hook = _ntff_profile_via_ctypes("/opt/axon/libaxon_pjrt.so")
        mod = types.ModuleType("antenv.axon_hooks")
        mod.get_axon_ntff_profile_hook = lambda: hook
        sys.modules["antenv.axon_hooks"] = mod
        import antenv
        antenv.axon_hooks = mod
    except Exception:
        pass


def to_bf16(a: np.ndarray) -> np.ndarray:
    return np.asarray(a, dtype=ml_dtypes.bfloat16)


def blocks_of(m):
    """Even m-block sizes: ceil(m/MB) blocks, sizes differing by <=1."""
    nb = (m + MB - 1) // MB
    base = m // nb
    rem = m - base * nb
    return [base + (1 if i < rem else 0) for i in range(nb)]


def build_nc(m_list, K, NC_N, psum_bufs=8, w_bufs=20, x_bufs=6, out_bufs=3):
    """SPMD program for per-segment (expert) valid row counts m_list."""
    KK = K // P
    KH = KK // 2
    NN = NC_N // P
    SM = sum(m_list)

    nc = bacc.Bacc("TRN2", target_bir_lowering=False, debug=False,
                   num_devices=N_CORES)
    n_blocks = sum(len(blocks_of(m)) for m in m_list)
    # x^T, packed per (segment, m-block): [128, KK*mbs] used cols
    xsw = nc.dram_tensor("xsw", [n_blocks * P, KK * MB], mybir.dt.bfloat16,
                         kind="ExternalInput").ap()
    # weights, packed per (segment, nn): row ((seg*NN + nn)*P + p),
    # col kk*P + j  =  w[seg, nn*P + j, kk*P + p]
    wsw = nc.dram_tensor("wsw", [len(m_list) * NN * P, KK * P],
                         mybir.dt.bfloat16, kind="ExternalInput").ap()
    # output, transposed+interleaved: [p, nn*SM + m] = out[m, nn*P + p]
    out = nc.dram_tensor("out", [P, NN * SM], mybir.dt.bfloat16,
                         kind="ExternalOutput").ap()
    out3 = out.rearrange("p (a m) -> p a m", a=NN)

    with tile.TileContext(nc) as tc:
        with tc.tile_pool(name="wp", bufs=w_bufs) as wp, \
             tc.tile_pool(name="xp", bufs=x_bufs) as xp, \
             tc.tile_pool(name="op", bufs=out_bufs) as op, \
             tc.tile_pool(name="pp", bufs=psum_bufs, space="PSUM") as pp, \
             tc.tile_pool(name="wu", bufs=1) as wu:
            # PE warmup: dummy bf16 matmuls spanning the initial DMA wait
            # keep the HAM activity monitor engaged so the PE clock is at
            # 2.4 GHz when the first real tiles land (needs >=4us of
            # continuous matmul to fully ramp).
            wa_r = wu.tile([P, MB], mybir.dt.bfloat16, tag="war")
            nc.gpsimd.memset(wa_r[:], 0.0)
            wpss = [pp.tile([P, MB], mybir.dt.float32, tag="ps",
                            name="wps") for _ in range(4)]
            for i in range(8):
                nc.tensor.matmul(wpss[i % 4][:], wa_r[:, :P], wa_r[:],
                                 start=True, stop=True)
            blk = 0
            col0 = 0
            KQ = KK // 4
            n_segs = len(m_list)

            def load_x(mbs, blk, first):
                # x as 4 quarter-K tiles: the first matmul only waits
                # for a quarter of the block's x, not half.  The very
                # first block is split across all three DMA queues.
                xts = [xp.tile([P, KQ * MB], mybir.dt.bfloat16,
                               tag=f"x{q}", name=f"xt{q}")
                       for q in range(4)]
                engs = [nc.scalar, nc.sync, nc.gpsimd, nc.scalar] if first \
                    else [nc.scalar] * 4
                for q in range(4):
                    engs[q].dma_start(
                        out=xts[q][:, :KQ * mbs],
                        in_=xsw[blk * P:(blk + 1) * P,
                                q * KQ * mbs:(q + 1) * KQ * mbs])
                return xts

            for seg, m in enumerate(m_list):
                w_ts = []
                blocks = blocks_of(m)
                xts0 = None
                for nn in range(NN):
                    w_t = wp.tile([P, KK * P], mybir.dt.bfloat16, tag="w")
                    r0 = (seg * NN + nn) * P
                    if seg == 0 and nn == 0:
                        # critical first tile: split across all three DMA
                        # queues so the very first matmul starts ~5us in
                        engs4 = [nc.sync, nc.scalar, nc.gpsimd, nc.sync]
                        qc = (KK * P) // 4
                        for q in range(4):
                            engs4[q].dma_start(
                                out=w_t[:, q * qc:(q + 1) * qc],
                                in_=wsw[r0:r0 + P, q * qc:(q + 1) * qc])
                        # first block's x goes out right after the first
                        # weight tile, ahead of the remaining 3.5MB of
                        # first-expert weights
                        xts0 = load_x(blocks[0], blk, True)
                        blk += 1
                    else:
                        # first expert: odd nn tiles ride the (idle) gpsimd
                        # queue so the 4MB expert load halves in latency
                        eng = nc.gpsimd if (seg == 0 and nn % 2 == 1) \
                            else nc.sync
                        eng.dma_start(out=w_t[:], in_=wsw[r0:r0 + P, :])
                    w_ts.append(w_t)
                for bi, mbs in enumerate(blocks):
                    if seg == 0 and bi == 0:
                        xts = xts0
                    else:
                        xts = load_x(mbs, blk, False)
                        blk += 1
                    # last two segments: per-nn output DMAs spread across
                    # all queues overlap the final casts so the kernel
                    # tail is one small DMA, not one big consolidated one
                    tail_seg = seg >= n_segs - 2
                    o_t = op.tile([P, NN * MB], mybir.dt.bfloat16, tag="o")
                    tail_engs = [nc.gpsimd, nc.sync, nc.scalar, nc.gpsimd]
                    for nn in range(NN):
                        ps = pp.tile([P, MB], mybir.dt.float32, tag="ps",
                                     name="ps")
                        for kk in range(KK):
                            xt = xts[kk // KQ]
                            j = kk % KQ
                            nc.tensor.matmul(
                                ps[:, :mbs],
                                w_ts[nn][:, kk * P:(kk + 1) * P],
                                xt[:, j * mbs:(j + 1) * mbs],
                                start=(kk == 0), stop=(kk == KK - 1))
                        nc.vector.tensor_copy(
                            o_t[:, nn * mbs:(nn + 1) * mbs], ps[:, :mbs])
                        if tail_seg:
                            tail_engs[nn % 4].dma_start(
                                out=out3[:, nn, col0:col0 + mbs],
                                in_=o_t[:, nn * mbs:(nn + 1) * mbs])
                    if not tail_seg:
                        nc.gpsimd.dma_start(
                            out=out3[:, :, col0:col0 + mbs],
                            in_=o_t[:, :NN * mbs])
                    col0 += mbs
    nc.compile()
    return nc


_NC_CACHE = {}


def get_nc(m_list, K, NC_N, **kw):
    key = (tuple(m_list), K, NC_N, tuple(sorted(kw.items())))
    if key not in _NC_CACHE:
        _NC_CACHE[key] = build_nc(m_list, K, NC_N, **kw)
    return _NC_CACHE[key]


# ----------------------------------------------------------------------
# Strassen (1 level) variant: 7/8 of the PE work.
#
# Per expert (m rows, K=2048, per-core N slice 1024):
#   split M -> 2 halves of mh=ceil(m/2), K -> 2x1024, N -> 2x512.
#   7 products P_p = Ac_p @ Wc_p^T, each [mh, 1024] x [512, 1024]^T,
#   recombined into quadrants C11/C12/C21/C22 by vector+gpsimd.
#   x-combos (5 adds/subs per m-block) and w-combos (5 per expert-nn)
#   are computed on-device by the otherwise idle vector/gpsimd engines
#   so HBM traffic stays at the baseline 81MB/core.
# Queues: sync = raw w quadrants, scalar = raw x quadrants, gpsimd = out.
# PSUM: 7 banks per (nn', m-block) group + 1 spare for pipelining.
# ----------------------------------------------------------------------
MBS = 256        # Strassen m-block cap (SBUF-pressure bound)
KH = 8           # k-tiles per K-half
NH = 4           # n-tiles per N-half

# Winograd-Strassen operands.  Products (0-based banks):
#   ban0 = A11 B11t   ban1 = A12 B12t   ban2 = S4 B22t
#   ban3 = A22 T4't (= -P4_classic, sign folded)   ban4 = S1 T1t
#   ban5 = S2 T2t     ban6 = S3 T3t
# x-combos: S1 = A21+A22, S2 = S1-A11, S3 = A11-A21, S4 = A12-S2
# w-combos: T1 = B21-B11, T2 = B22-T1, T3 = B22-B21, T4' = B12-T2
# recombine (all adds): C11 = ban0+ban1 ; U2 = ban0+ban5 ; U3 = U2+ban6
#   U4 = U2+ban4 ; C12 = U4+ban2 ; C21 = U3+ban3 ; C22 = U3+ban4


def blocks_of_s(mh):
    nb = (mh + MBS - 1) // MBS
    base = mh // nb
    rem = mh - base * nb
    return [base + (1 if i < rem else 0) for i in range(nb)]


def build_nc_strassen(m_list, w_bufs=38, x_bufs=8, xc_bufs=6):
    """m_list: full per-expert row counts (processing order)."""
    mh_list = [(m + 1) // 2 for m in m_list]
    n_blocks = sum(len(blocks_of_s(mh)) for mh in mh_list)
    out_cols = 16 * sum(mh_list)

    nc = bacc.Bacc("TRN2", target_bir_lowering=False, debug=False,
                   num_devices=N_CORES)
    # raw x quadrants per (expert, block, quad): [128, KH*mbs] used cols
    xsw = nc.dram_tensor("xsw", [n_blocks * 4 * P, KH * MBS],
                         mybir.dt.bfloat16, kind="ExternalInput").ap()
    # raw w quadrants per (expert, nn', quad): [128, KH*128]
    wsw = nc.dram_tensor("wsw", [len(m_list) * NH * 4 * P, KH * P],
                         mybir.dt.bfloat16, kind="ExternalInput").ap()
    out = nc.dram_tensor("out", [P, out_cols], mybir.dt.bfloat16,
                         kind="ExternalOutput").ap()

    with tile.TileContext(nc) as tc:
        with tc.tile_pool(name="wrp", bufs=10) as wrp, \
             tc.tile_pool(name="wcp", bufs=w_bufs) as wcp, \
             tc.tile_pool(name="xrp", bufs=x_bufs) as xrp, \
             tc.tile_pool(name="xcp", bufs=xc_bufs) as xcp, \
             tc.tile_pool(name="opo", bufs=4) as opo, \
             tc.tile_pool(name="tpv", bufs=2) as tpv, \
             tc.tile_pool(name="ppp", bufs=8, space="PSUM") as ppp, \
             tc.tile_pool(name="wup", bufs=1) as wup:
            # PE warmup (HAM ramp) while first DMAs land.  12 matmuls
            # (~2.6us) end just before the first real matmul (~7.3us):
            # 8 left a ~2.8us PE idle that reset the HAM continuity
            # window right before the real work began.
            wa_r = wup.tile([P, MB], mybir.dt.bfloat16, tag="war")
            nc.gpsimd.memset(wa_r[:], 0.0)
            wpss = [ppp.tile([P, MB], mybir.dt.float32, tag="ps",
                             name="wps") for _ in range(4)]
            for i in range(12):
                nc.tensor.matmul(wpss[i % 4][:], wa_r[:, :P], wa_r[:],
                                 start=True, stop=True)

            def load_w_nn(ei, nn, first):
                """DMA 4 raw quadrant tiles for one nn', build its 7
                stationary operands (4 combos on gpsimd + 3 raws).
                Winograd: T1=B21-B11 T2=B22-T1 T3=B22-B21 T4'=B12-T2."""
                raws = []
                for q in range(4):
                    r0 = ((ei * NH + nn) * 4 + q) * P
                    keep = q in (0, 1, 3)     # B11, B12, B22 stay
                    pool = wcp if keep else wrp
                    wr = pool.tile([P, KH * P], mybir.dt.bfloat16,
                                   tag="wc" if keep else "wt",
                                   name="wr")
                    if first and nn == 0:
                        engs = [nc.sync, nc.scalar, nc.gpsimd, nc.sync]
                        engs[q].dma_start(out=wr[:],
                                          in_=wsw[r0:r0 + P, :])
                    elif ei == 0:
                        # first expert: burst raw w across all 3 queues
                        # so pair-1 isn't starved at t~18us
                        engs3 = [nc.sync, nc.gpsimd, nc.scalar]
                        engs3[(nn * 4 + q) % 3].dma_start(
                            out=wr[:], in_=wsw[r0:r0 + P, :])
                    elif ei == 1:
                        eng = nc.gpsimd if (nn * 4 + q) % 2 else nc.sync
                        eng.dma_start(out=wr[:], in_=wsw[r0:r0 + P, :])
                    else:
                        nc.sync.dma_start(out=wr[:],
                                          in_=wsw[r0:r0 + P, :])
                    raws.append(wr)
                b11, b12, b21, b22 = raws
                cs = [wcp.tile([P, KH * P], mybir.dt.bfloat16, tag="wc",
                               name="wcc") for _ in range(4)]
                t1, t2, t3, t4 = cs
                # vector, not gpsimd: gpsimd is ~2.3us/op and would block
                # the out-DMA triggers queued behind it
                nc.vector.tensor_sub(t1[:], b21[:], b11[:])
                nc.vector.tensor_sub(t2[:], b22[:], t1[:])
                nc.vector.tensor_sub(t3[:], b22[:], b21[:])
                nc.vector.tensor_sub(t4[:], b12[:], t2[:])   # -T4
                # stationary operand of product p
                return [b11, b12, b22, t4, t1, t2, t3]

            blk = 0
            col0 = 0
            NPAIR = NH // 2
            for ei, m in enumerate(m_list):
                mh = mh_list[ei]
                if ei == 0:
                    # head: first nn' weights, then first x block, then
                    # the rest of the first expert's weights
                    wops = [load_w_nn(0, 0, True)]
                else:
                    wops = [load_w_nn(ei, nn, False) for nn in range(NH)]
                b0 = 0
                for bi, mbs in enumerate(blocks_of_s(mh)):
                    first_blk = ei == 0 and bi == 0
                    raws = []
                    for q in range(4):
                        xr = xrp.tile([P, KH * MBS], mybir.dt.bfloat16,
                                      tag="xr", name="xr")
                        if ei == 0 and bi <= 2:
                            # expert-0's x blocks otherwise serialize on
                            # the scalar queue (~50us for 4 blocks) while
                            # sync/gpsimd carry only 4.2MB of weights --
                            # that starved pair-0 of block 1 for ~9us and
                            # tripped a 10us K=4 HAM window.  Rotate the
                            # first three blocks across all three queues.
                            rot = [nc.scalar, nc.sync, nc.gpsimd]
                            engs = [rot[(bi + i) % 3] for i in range(4)]
                        else:
                            engs = [nc.scalar] * 4
                        engs[q].dma_start(
                            out=xr[:, :KH * mbs],
                            in_=xsw[(blk * 4 + q) * P:(blk * 4 + q + 1) * P,
                                    :KH * mbs])
                        raws.append(xr)
                    blk += 1
                    a11, a12, a21, a22 = raws
                    w = KH * mbs
                    cs = [xcp.tile([P, KH * MBS], mybir.dt.bfloat16,
                                   tag="xc", name="xcc") for _ in range(4)]
                    s1, s2, s3, s4 = cs
                    nc.vector.tensor_add(s1[:, :w], a21[:, :w], a22[:, :w])
                    nc.vector.tensor_sub(s2[:, :w], s1[:, :w], a11[:, :w])
                    nc.vector.tensor_sub(s3[:, :w], a11[:, :w], a21[:, :w])
                    nc.vector.tensor_sub(s4[:, :w], a12[:, :w], s2[:, :w])
                    xops = [a11, a12, s4, a22, s1, s2, s3]
                    if first_blk:
                        wops.extend(load_w_nn(0, nn, False)
                                    for nn in range(1, NH))
                    for npair in range(NPAIR):
                        # two nn' share each PSUM bank: [:, h*mbs:...]
                        ban = []
                        for p in range(7):
                            ps = ppp.tile([P, MB], mybir.dt.float32,
                                          tag="ps", name="ps")
                            for h in (0, 1):
                                wt = wops[npair * 2 + h][p]
                                for kk in range(KH):
                                    nc.tensor.matmul(
                                        ps[:, h * mbs:h * mbs + mbs],
                                        wt[:, kk * P:(kk + 1) * P],
                                        xops[p][:, kk * mbs:(kk + 1) * mbs],
                                        start=(h == 0 and kk == 0),
                                        stop=(h == 1 and kk == KH - 1))
                            ban.append(ps)
                        # all-add Winograd recombination, <=1 PSUM per op
                        o_t = opo.tile([P, 8 * MBS], mybir.dt.bfloat16,
                                       tag="o", name="o")
                        s = slice(0, 2 * mbs)
                        # bf16 intermediates: ~45% less DVE byte traffic,
                        # keeps the HAM power throttle off the PE
                        # (rel err 9.1e-3 vs 8.3e-3 fp32, budget 2e-2)
                        cp = tpv.tile([P, 2 * MBS], mybir.dt.bfloat16,
                                      tag="cp", name="cp")
                        u2 = tpv.tile([P, 2 * MBS], mybir.dt.bfloat16,
                                      tag="u2", name="u2")
                        u3 = tpv.tile([P, 2 * MBS], mybir.dt.bfloat16,
                                      tag="u3", name="u3")
                        u4 = tpv.tile([P, 2 * MBS], mybir.dt.bfloat16,
                                      tag="u4", name="u4")
                        CPY = mybir.ActivationFunctionType.Copy
                        nc.scalar.activation(cp[:, s], ban[0][:, s], CPY)
                        nc.vector.tensor_add(o_t[:, 0:2 * mbs],
                                             ban[1][:, s], cp[:, s])  # C11
                        nc.vector.tensor_add(u2[:, s], ban[5][:, s],
                                             cp[:, s])
                        nc.vector.tensor_add(u3[:, s], ban[6][:, s],
                                             u2[:, s])
                        nc.vector.tensor_add(u4[:, s], ban[4][:, s],
                                             u2[:, s])
                        nc.vector.tensor_add(o_t[:, 2 * mbs:4 * mbs],
                                             ban[2][:, s], u4[:, s])  # C12
                        nc.vector.tensor_add(o_t[:, 4 * mbs:6 * mbs],
                                             ban[3][:, s], u3[:, s])  # C21
                        nc.vector.tensor_add(o_t[:, 6 * mbs:8 * mbs],
                                             ban[4][:, s], u3[:, s])  # C22
                        if ei == len(m_list) - 1:
                            # tail: last expert's outputs ride all three
                            # queues in halves so the final DMA is small
                            h4 = 4 * mbs
                            t_engs = [nc.sync, nc.scalar]
                            for ti in range(2):
                                t_engs[ti].dma_start(
                                    out=out[:, col0 + ti * h4:
                                            col0 + (ti + 1) * h4],
                                    in_=o_t[:, ti * h4:(ti + 1) * h4])
                        else:
                            nc.gpsimd.dma_start(
                                out=out[:, col0:col0 + 8 * mbs],
                                in_=o_t[:, :8 * mbs])
                        col0 += 8 * mbs
                    b0 += mbs
    nc.compile()
    return nc


def pack_x_strassen(x_padded, order, m_all, K):
    parts = []
    for e in order:
        m = m_all[e]
        mh = (m + 1) // 2
        xp2 = np.zeros((2 * mh, K), dtype=np.float32)
        xp2[:m] = x_padded[e, :m, :]
        b0 = 0
        for mbs in blocks_of_s(mh):
            for r in (0, 1):
                for g in (0, 1):
                    q = xp2[r * mh + b0:r * mh + b0 + mbs,
                            g * (K // 2):(g + 1) * (K // 2)]
                    b = q.T.reshape(KH, P, mbs).transpose(1, 0, 2)
                    row = np.zeros((P, KH * MBS), dtype=np.float32)
                    row[:, :KH * mbs] = b.reshape(P, KH * mbs)
                    parts.append(row)
            b0 += mbs
    return to_bf16(np.concatenate(parts, axis=0))


def pack_w_strassen(stacked_weights, order, c, NC_N, K):
    parts = []
    for e in order:
        blkw = stacked_weights[e, c * NC_N:(c + 1) * NC_N, :]  # [1024, K]
        for nn in range(NH):
            for (h, g) in ((0, 0), (0, 1), (1, 0), (1, 1)):
                q = blkw[h * 512 + nn * P:h * 512 + (nn + 1) * P,
                         g * (K // 2):(g + 1) * (K // 2)]  # [128, 1024]
                # stationary: [p, kk*128 + j] = q[j, kk*128 + p]
                a = q.reshape(P, KH, P).transpose(2, 1, 0)
                parts.append(np.ascontiguousarray(a).reshape(P, KH * P))
    return to_bf16(np.concatenate(parts, axis=0))


def kernel_strassen(x_padded, stacked_weights, m_sizes):
    global LAST_RESULT
    x_padded = np.ascontiguousarray(np.asarray(x_padded, dtype=np.float32))
    stacked_weights = np.ascontiguousarray(
        np.asarray(stacked_weights, dtype=np.float32))
    E, MAX_M, K = x_padded.shape
    N = stacked_weights.shape[1]
    NC_N = N // N_CORES
    m_all = [int(min(max(int(mm), 0), MAX_M))
             for mm in np.asarray(m_sizes).astype(np.int64)]

    out_full = np.zeros((E, MAX_M, N), dtype=np.float32)
    order = [e for e in range(E) if m_all[e] > 0]
    if not order:
        return out_full
    order.sort(key=lambda e: -m_all[e])
    # interleave big/small so small experts' w prefetch hides under the
    # preceding big expert's long PE stretch (small experts are w-bound)
    desc = order
    inter = []
    i, j = 0, len(desc) - 1
    while i <= j:
        inter.append(desc[i])
        if j > i:
            inter.append(desc[j])
        i += 1
        j -= 1
    order = inter
    m_list = [m_all[e] for e in order]

    _install_profile_shim()
    key = ("strassen", tuple(m_list))
    if key not in _NC_CACHE:
        _NC_CACHE[key] = build_nc_strassen(m_list)
    nc = _NC_CACHE[key]

    xsw = pack_x_strassen(x_padded, order, m_all, K)
    in_maps = [{"xsw": xsw,
                "wsw": pack_w_strassen(stacked_weights, order, c, NC_N, K)}
               for c in range(N_CORES)]
    res = run_bass_kernel_spmd(nc, in_maps, list(range(N_CORES)))
    LAST_RESULT = res

    for c in range(N_CORES):
        o = np.asarray(res.results[c]["out"]).astype(np.float32)
        col = 0
        for i, e in enumerate(order):
            m = m_list[i]
            mh = (m + 1) // 2
            b0 = 0
            for mbs in blocks_of_s(mh):
                for npair in range(NH // 2):
                    for (r, h) in ((0, 0), (0, 1), (1, 0), (1, 1)):
                        for hh in (0, 1):
                            nn = npair * 2 + hh
                            t = o[:, col:col + mbs]  # [p, j]
                            r0 = r * mh + b0
                            r1 = min(r0 + mbs, m)
                            if r1 > r0:
                                n0 = c * NC_N + h * 512 + nn * P
                                out_full[e, r0:r1, n0:n0 + P] = \
                                    t[:, :r1 - r0].T
                            col += mbs
                b0 += mbs
    return out_full


def pack_x(x_padded, order, m_all, K):
    """x^T packed per (expert, m-block): [128, KK*mbs] rows, padded to
    the fixed [P, KK*MB] dram row width."""
    KK = K // P
    parts = []
    for e in order:
        m = m_all[e]
        mb0 = 0
        for mbs in blocks_of(m):
            b = x_padded[e, mb0:mb0 + mbs, :].T        # [K, mbs]
            b = b.reshape(KK, P, mbs).transpose(1, 0, 2)  # [P, KK, mbs]
            row = np.zeros((P, KK * MB), dtype=np.float32)
            row[:, :KK * mbs] = b.reshape(P, KK * mbs)
            parts.append(row)
            mb0 += mbs
    return to_bf16(np.concatenate(parts, axis=0))


def pack_w(stacked_weights, order, c, NC_N, K):
    """Weights per (expert, nn): [128, KK*P] with
    [p, kk*P+j] = w[e, c*NC_N + nn*P + j, kk*P + p]."""
    KK = K // P
    NN = NC_N // P
    parts = []
    for e in order:
        blk = stacked_weights[e, c * NC_N:(c + 1) * NC_N, :]  # [NC_N, K]
        a = blk.reshape(NN, P, KK, P).transpose(0, 3, 2, 1)   # [nn, p, kk, j]
        parts.append(np.ascontiguousarray(a).reshape(NN * P, KK * P))
    return to_bf16(np.concatenate(parts, axis=0))


def kernel(x_padded, stacked_weights, m_sizes):
    import os
    if os.environ.get("BASS_IMPL", "strassen") == "strassen":
        return kernel_strassen(x_padded, stacked_weights, m_sizes)
    return kernel_base(x_padded, stacked_weights, m_sizes)


def kernel_base(x_padded, stacked_weights, m_sizes):
    global LAST_RESULT
    x_padded = np.ascontiguousarray(np.asarray(x_padded, dtype=np.float32))
    stacked_weights = np.ascontiguousarray(
        np.asarray(stacked_weights, dtype=np.float32))
    E, MAX_M, K = x_padded.shape
    N = stacked_weights.shape[1]
    NC_N = N // N_CORES
    NN = NC_N // P
    m_all = [int(min(max(int(mm), 0), MAX_M))
             for mm in np.asarray(m_sizes).astype(np.int64)]

    out_full = np.zeros((E, MAX_M, N), dtype=np.float32)
    order = [e for e in range(E) if m_all[e] > 0]
    if not order:
        return out_full
    # descending size: the big first expert hides its own weight load,
    # and the small trailing experts ride 3-expert weight prefetch
    order.sort(key=lambda e: -m_all[e])
    m_list = [m_all[e] for e in order]
    SM = sum(m_list)

    _install_profile_shim()
    nc = get_nc(m_list, K, NC_N)

    xsw = pack_x(x_padded, order, m_all, K)
    in_maps = [{"xsw": xsw,
                "wsw": pack_w(stacked_weights, order, c, NC_N, K)}
               for c in range(N_CORES)]

    res = run_bass_kernel_spmd(nc, in_maps, list(range(N_CORES)))
    LAST_RESULT = res

    for c in range(N_CORES):
        o = np.asarray(res.results[c]["out"]).astype(np.float32)  # [P, NN*SM]
        outT = o.reshape(P, NN, SM).transpose(1, 0, 2).reshape(NC_N, SM)
        col = 0
        for i, e in enumerate(order):
            m = m_list[i]
            out_full[e, :m, c * NC_N:(c + 1) * NC_N] = outT[:, col:col + m].T
            col += m
    return out_full



# revision 37
# speedup vs baseline: 1.0409x; 1.0283x over previous
"""Grouped-GEMM (MoE expert FFN) kernel for 8 Trainium2 NeuronCores.

Problem: out[e, m, n] = sum_k x[e, m, k] * w[e, n, k] for m < m_sizes[e],
         zero elsewhere.  E=8, MAX_M=2048, K=2048, N=8192, fp32.

Default implementation: 1-level Winograd-Strassen (kernel_strassen,
~430us) -- 7/8 of the bf16 PE work of the direct kernel (kernel_base,
~445us, kept as BASS_IMPL=base fallback).

Shared structure
----------------------------------------------------------------------
* N-split sharding: every core computes ALL experts against its own
  (N/8)=1024-wide column slice of each expert's weights (perfect load
  balance; each weight element read once fleet-wide).
* exact-m: the moving free dim is the token count, so each expert does
  exactly m_e rows of PE work, zero padding (Strassen: ceil(m/2)).
* bf16 operands at 1 PE cycle/row; fp32 PSUM accumulate.

Winograd-Strassen specifics (see build_nc_strassen)
----------------------------------------------------------------------
* Per expert: M->2 x K->2x1024 x N->2x512 split, 7 products
  recombined with the all-ADD Winograd schedule (P4's sign folded into
  its weight combo), so every recombination op has <=1 PSUM operand.
* x-combos + w-combos computed on the DVE (vector) from raw quadrant
  streams -- HBM traffic stays at the direct kernel's 81MB/core.
  GpSimd must not do this work: it cannot read PSUM, costs ~2.3us/op,
  and blocks the out-DMA triggers queued behind it.
* Two nn' tiles share each PSUM bank ([128, 2*256] fp32) so 7 banks
  hold a product group and recombination ops are few and wide.
* bf16 recombination intermediates: the chip's activity monitor
  (HAM) power-throttles the PE to ~81% for ~160us when PE + DVE + DMA
  all run hot; bf16 intermediates cut DVE byte traffic enough to keep
  the throttle off (429960ns vs 445288ns fp32, rel err 1.26e-2 vs
  tolerance 2e-2).
* Queues: sync = raw w quadrants, scalar = raw x quadrants,
  gpsimd = output; first expert's tiles burst across all three.
* Expert order interleaves big/small so small (w-DMA-bound) experts'
  weight streams prefetch under the preceding big expert's PE time.
"""
import sys
import types

import ml_dtypes
import numpy as np

import concourse.bass as bass
import concourse.tile as tile
from concourse import bacc, mybir
from concourse.bass_utils import run_bass_kernel_spmd

P = 128          # partition dim / k-tile
N_CORES = 8
MB = 512         # max moving rows per matmul (one PSUM bank of fp32)

LAST_RESULT = None   # BassKernelResults of the most recent run (for tests)


def _install_profile_shim():
    """The agent image's antenv stub lacks axon_hooks; provide it so
    BASS_TRACE=1 profiling works instead of crashing."""
    if "antenv.axon_hooks" in sys.modules:
        return
    try:
        from trn_agent_boot.trn_boot import _ntff_profile_via_ctypes
        hook = _ntff_profile_via_ctypes("/opt/axon/libaxon_pjrt.so")
        mod = types.ModuleType("antenv.axon_hooks")
        mod.get_axon_ntff_profile_hook = lambda: hook
        sys.modules["antenv.axon_hooks"] = mod
        import antenv
        antenv.axon_hooks = mod
    except Exception:
        pass


def to_bf16(a: np.ndarray) -> np.ndarray:
    return np.asarray(a, dtype=ml_dtypes.bfloat16)


def blocks_of(m):
    """Even m-block sizes: ceil(m/MB) blocks, sizes differing by <=1."""
    nb = (m + MB - 1) // MB
    base = m // nb
    rem = m - base * nb
    return [base + (1 if i < rem else 0) for i in range(nb)]


def build_nc(m_list, K, NC_N, psum_bufs=8, w_bufs=20, x_bufs=6, out_bufs=3):
    """SPMD program for per-segment (expert) valid row counts m_list."""
    KK = K // P
    KH = KK // 2
    NN = NC_N // P
    SM = sum(m_list)

    nc = bacc.Bacc("TRN2", target_bir_lowering=False, debug=False,
                   num_devices=N_CORES)
    n_blocks = sum(len(blocks_of(m)) for m in m_list)
    # x^T, packed per (segment, m-block): [128, KK*mbs] used cols
    xsw = nc.dram_tensor("xsw", [n_blocks * P, KK * MB], mybir.dt.bfloat16,
                         kind="ExternalInput").ap()
    # weights, packed per (segment, nn): row ((seg*NN + nn)*P + p),
    # col kk*P + j  =  w[seg, nn*P + j, kk*P + p]
    wsw = nc.dram_tensor("wsw", [len(m_list) * NN * P, KK * P],
                         mybir.dt.bfloat16, kind="ExternalInput").ap()
    # output, transposed+interleaved: [p, nn*SM + m] = out[m, nn*P + p]
    out = nc.dram_tensor("out", [P, NN * SM], mybir.dt.bfloat16,
                         kind="ExternalOutput").ap()
    out3 = out.rearrange("p (a m) -> p a m", a=NN)

    with tile.TileContext(nc) as tc:
        with tc.tile_pool(name="wp", bufs=w_bufs) as wp, \
             tc.tile_pool(name="xp", bufs=x_bufs) as xp, \
             tc.tile_pool(name="op", bufs=out_bufs) as op, \
             tc.tile_pool(name="pp", bufs=psum_bufs, space="PSUM") as pp, \
             tc.tile_pool(name="wu", bufs=1) as wu:
            # PE warmup: dummy bf16 matmuls spanning the initial DMA wait
            # keep the HAM activity monitor engaged so the PE clock is at
            # 2.4 GHz when the first real tiles land (needs >=4us of
            # continuous matmul to fully ramp).
            wa_r = wu.tile([P, MB], mybir.dt.bfloat16, tag="war")
            nc.gpsimd.memset(wa_r[:], 0.0)
            wpss = [pp.tile([P, MB], mybir.dt.float32, tag="ps",
                            name="wps") for _ in range(4)]
            for i in range(8):
                nc.tensor.matmul(wpss[i % 4][:], wa_r[:, :P], wa_r[:],
                                 start=True, stop=True)
            blk = 0
            col0 = 0
            KQ = KK // 4
            n_segs = len(m_list)

            def load_x(mbs, blk, first):
                # x as 4 quarter-K tiles: the first matmul only waits
                # for a quarter of the block's x, not half.  The very
                # first block is split across all three DMA queues.
                xts = [xp.tile([P, KQ * MB], mybir.dt.bfloat16,
                               tag=f"x{q}", name=f"xt{q}")
                       for q in range(4)]
                engs = [nc.scalar, nc.sync, nc.gpsimd, nc.scalar] if first \
                    else [nc.scalar] * 4
                for q in range(4):
                    engs[q].dma_start(
                        out=xts[q][:, :KQ * mbs],
                        in_=xsw[blk * P:(blk + 1) * P,
                                q * KQ * mbs:(q + 1) * KQ * mbs])
                return xts

            for seg, m in enumerate(m_list):
                w_ts = []
                blocks = blocks_of(m)
                xts0 = None
                for nn in range(NN):
                    w_t = wp.tile([P, KK * P], mybir.dt.bfloat16, tag="w")
                    r0 = (seg * NN + nn) * P
                    if seg == 0 and nn == 0:
                        # critical first tile: split across all three DMA
                        # queues so the very first matmul starts ~5us in
                        engs4 = [nc.sync, nc.scalar, nc.gpsimd, nc.sync]
                        qc = (KK * P) // 4
                        for q in range(4):
                            engs4[q].dma_start(
                                out=w_t[:, q * qc:(q + 1) * qc],
                                in_=wsw[r0:r0 + P, q * qc:(q + 1) * qc])
                        # first block's x goes out right after the first
                        # weight tile, ahead of the remaining 3.5MB of
                        # first-expert weights
                        xts0 = load_x(blocks[0], blk, True)
                        blk += 1
                    else:
                        # first expert: odd nn tiles ride the (idle) gpsimd
                        # queue so the 4MB expert load halves in latency
                        eng = nc.gpsimd if (seg == 0 and nn % 2 == 1) \
                            else nc.sync
                        eng.dma_start(out=w_t[:], in_=wsw[r0:r0 + P, :])
                    w_ts.append(w_t)
                for bi, mbs in enumerate(blocks):
                    if seg == 0 and bi == 0:
                        xts = xts0
                    else:
                        xts = load_x(mbs, blk, False)
                        blk += 1
                    # last two segments: per-nn output DMAs spread across
                    # all queues overlap the final casts so the kernel
                    # tail is one small DMA, not one big consolidated one
                    tail_seg = seg >= n_segs - 2
                    o_t = op.tile([P, NN * MB], mybir.dt.bfloat16, tag="o")
                    tail_engs = [nc.gpsimd, nc.sync, nc.scalar, nc.gpsimd]
                    for nn in range(NN):
                        ps = pp.tile([P, MB], mybir.dt.float32, tag="ps",
                                     name="ps")
                        for kk in range(KK):
                            xt = xts[kk // KQ]
                            j = kk % KQ
                            nc.tensor.matmul(
                                ps[:, :mbs],
                                w_ts[nn][:, kk * P:(kk + 1) * P],
                                xt[:, j * mbs:(j + 1) * mbs],
                                start=(kk == 0), stop=(kk == KK - 1))
                        nc.vector.tensor_copy(
                            o_t[:, nn * mbs:(nn + 1) * mbs], ps[:, :mbs])
                        if tail_seg:
                            tail_engs[nn % 4].dma_start(
                                out=out3[:, nn, col0:col0 + mbs],
                                in_=o_t[:, nn * mbs:(nn + 1) * mbs])
                    if not tail_seg:
                        nc.gpsimd.dma_start(
                            out=out3[:, :, col0:col0 + mbs],
                            in_=o_t[:, :NN * mbs])
                    col0 += mbs
    nc.compile()
    return nc


_NC_CACHE = {}


def get_nc(m_list, K, NC_N, **kw):
    key = (tuple(m_list), K, NC_N, tuple(sorted(kw.items())))
    if key not in _NC_CACHE:
        _NC_CACHE[key] = build_nc(m_list, K, NC_N, **kw)
    return _NC_CACHE[key]


# ----------------------------------------------------------------------
# Strassen (1 level) variant: 7/8 of the PE work.
#
# Per expert (m rows, K=2048, per-core N slice 1024):
#   split M -> 2 halves of mh=ceil(m/2), K -> 2x1024, N -> 2x512.
#   7 products P_p = Ac_p @ Wc_p^T, each [mh, 1024] x [512, 1024]^T,
#   recombined into quadrants C11/C12/C21/C22 by vector+gpsimd.
#   x-combos (5 adds/subs per m-block) and w-combos (5 per expert-nn)
#   are computed on-device by the otherwise idle vector/gpsimd engines
#   so HBM traffic stays at the baseline 81MB/core.
# Queues: sync = raw w quadrants, scalar = raw x quadrants, gpsimd = out.
# PSUM: 7 banks per (nn', m-block) group + 1 spare for pipelining.
# ----------------------------------------------------------------------
MBS = 256        # Strassen m-block cap (SBUF-pressure bound)
KH = 8           # k-tiles per K-half
NH = 4           # n-tiles per N-half

# Winograd-Strassen operands.  Products (0-based banks):
#   ban0 = A11 B11t   ban1 = A12 B12t   ban2 = S4 B22t
#   ban3 = A22 T4't (= -P4_classic, sign folded)   ban4 = S1 T1t
#   ban5 = S2 T2t     ban6 = S3 T3t
# x-combos: S1 = A21+A22, S2 = S1-A11, S3 = A11-A21, S4 = A12-S2
# w-combos: T1 = B21-B11, T2 = B22-T1, T3 = B22-B21, T4' = B12-T2
# recombine (all adds): C11 = ban0+ban1 ; U2 = ban0+ban5 ; U3 = U2+ban6
#   U4 = U2+ban4 ; C12 = U4+ban2 ; C21 = U3+ban3 ; C22 = U3+ban4


def blocks_of_s(mh):
    nb = (mh + MBS - 1) // MBS
    base = mh // nb
    rem = mh - base * nb
    return [base + (1 if i < rem else 0) for i in range(nb)]


def build_nc_strassen(m_list, w_bufs=42, x_bufs=8, xc_bufs=6):
    """m_list: full per-expert row counts (processing order)."""
    mh_list = [(m + 1) // 2 for m in m_list]
    n_blocks = sum(len(blocks_of_s(mh)) for mh in mh_list)
    out_cols = 16 * sum(mh_list)

    nc = bacc.Bacc("TRN2", target_bir_lowering=False, debug=False,
                   num_devices=N_CORES)
    # raw x quadrants per (expert, block, quad): [128, KH*mbs] used cols
    xsw = nc.dram_tensor("xsw", [n_blocks * 4 * P, KH * MBS],
                         mybir.dt.bfloat16, kind="ExternalInput").ap()
    # raw w quadrants per (expert, nn', quad): [128, KH*128]
    wsw = nc.dram_tensor("wsw", [len(m_list) * NH * 4 * P, KH * P],
                         mybir.dt.bfloat16, kind="ExternalInput").ap()
    out = nc.dram_tensor("out", [P, out_cols], mybir.dt.bfloat16,
                         kind="ExternalOutput").ap()

    with tile.TileContext(nc) as tc:
        with tc.tile_pool(name="wrp", bufs=10) as wrp, \
             tc.tile_pool(name="wcp", bufs=w_bufs) as wcp, \
             tc.tile_pool(name="xrp", bufs=x_bufs) as xrp, \
             tc.tile_pool(name="xcp", bufs=xc_bufs) as xcp, \
             tc.tile_pool(name="opo", bufs=4) as opo, \
             tc.tile_pool(name="tpv", bufs=2) as tpv, \
             tc.tile_pool(name="ppp", bufs=8, space="PSUM") as ppp, \
             tc.tile_pool(name="wup", bufs=1) as wup:
            # PE warmup (HAM ramp) while first DMAs land
            wa_r = wup.tile([P, MB], mybir.dt.bfloat16, tag="war")
            nc.gpsimd.memset(wa_r[:], 0.0)
            wpss = [ppp.tile([P, MB], mybir.dt.float32, tag="ps",
                             name="wps") for _ in range(4)]
            for i in range(8):
                nc.tensor.matmul(wpss[i % 4][:], wa_r[:, :P], wa_r[:],
                                 start=True, stop=True)

            def load_w_nn(ei, nn, first):
                """DMA 4 raw quadrant tiles for one nn', build its 7
                stationary operands (4 combos on gpsimd + 3 raws).
                Winograd: T1=B21-B11 T2=B22-T1 T3=B22-B21 T4'=B12-T2."""
                raws = []
                for q in range(4):
                    r0 = ((ei * NH + nn) * 4 + q) * P
                    keep = q in (0, 1, 3)     # B11, B12, B22 stay
                    pool = wcp if keep else wrp
                    wr = pool.tile([P, KH * P], mybir.dt.bfloat16,
                                   tag="wc" if keep else "wt",
                                   name="wr")
                    if first and nn == 0:
                        engs = [nc.sync, nc.scalar, nc.gpsimd, nc.sync]
                        engs[q].dma_start(out=wr[:],
                                          in_=wsw[r0:r0 + P, :])
                    elif ei == 0:
                        # first expert: burst raw w across all 3 queues
                        # so pair-1 isn't starved at t~18us
                        engs3 = [nc.sync, nc.gpsimd, nc.scalar]
                        engs3[(nn * 4 + q) % 3].dma_start(
                            out=wr[:], in_=wsw[r0:r0 + P, :])
                    elif ei == 1:
                        eng = nc.gpsimd if (nn * 4 + q) % 2 else nc.sync
                        eng.dma_start(out=wr[:], in_=wsw[r0:r0 + P, :])
                    else:
                        nc.sync.dma_start(out=wr[:],
                                          in_=wsw[r0:r0 + P, :])
                    raws.append(wr)
                b11, b12, b21, b22 = raws
                cs = [wcp.tile([P, KH * P], mybir.dt.bfloat16, tag="wc",
                               name="wcc") for _ in range(4)]
                t1, t2, t3, t4 = cs
                # vector, not gpsimd: gpsimd is ~2.3us/op and would block
                # the out-DMA triggers queued behind it
                nc.vector.tensor_sub(t1[:], b21[:], b11[:])
                nc.vector.tensor_sub(t2[:], b22[:], t1[:])
                nc.vector.tensor_sub(t3[:], b22[:], b21[:])
                nc.vector.tensor_sub(t4[:], b12[:], t2[:])   # -T4
                # stationary operand of product p
                return [b11, b12, b22, t4, t1, t2, t3]

            blk = 0
            col0 = 0
            NPAIR = NH // 2
            for ei, m in enumerate(m_list):
                mh = mh_list[ei]
                if ei == 0:
                    # head: first nn' weights, then first x block, then
                    # the rest of the first expert's weights
                    wops = [load_w_nn(0, 0, True)]
                else:
                    wops = [load_w_nn(ei, nn, False) for nn in range(NH)]
                b0 = 0
                for bi, mbs in enumerate(blocks_of_s(mh)):
                    first_blk = ei == 0 and bi == 0
                    raws = []
                    for q in range(4):
                        xr = xrp.tile([P, KH * MBS], mybir.dt.bfloat16,
                                      tag="xr", name="xr")
                        if ei == 0 and bi <= 2:
                            # expert-0's x blocks otherwise serialize on
                            # the scalar queue (~50us for 4 blocks) while
                            # sync/gpsimd carry only 4.2MB of weights --
                            # that starved pair-0 of block 1 for ~9us and
                            # tripped a 10us K=4 HAM window.  Rotate the
                            # first three blocks across all three queues.
                            rot = [nc.scalar, nc.sync, nc.gpsimd]
                            engs = [rot[(bi + i) % 3] for i in range(4)]
                        else:
                            engs = [nc.scalar] * 4
                        engs[q].dma_start(
                            out=xr[:, :KH * mbs],
                            in_=xsw[(blk * 4 + q) * P:(blk * 4 + q + 1) * P,
                                    :KH * mbs])
                        raws.append(xr)
                    blk += 1
                    a11, a12, a21, a22 = raws
                    w = KH * mbs
                    cs = [xcp.tile([P, KH * MBS], mybir.dt.bfloat16,
                                   tag="xc", name="xcc") for _ in range(4)]
                    s1, s2, s3, s4 = cs
                    nc.vector.tensor_add(s1[:, :w], a21[:, :w], a22[:, :w])
                    nc.vector.tensor_sub(s2[:, :w], s1[:, :w], a11[:, :w])
                    nc.vector.tensor_sub(s3[:, :w], a11[:, :w], a21[:, :w])
                    nc.vector.tensor_sub(s4[:, :w], a12[:, :w], s2[:, :w])
                    xops = [a11, a12, s4, a22, s1, s2, s3]
                    if first_blk:
                        wops.extend(load_w_nn(0, nn, False)
                                    for nn in range(1, NH))
                    for npair in range(NPAIR):
                        # two nn' share each PSUM bank: [:, h*mbs:...]
                        ban = []
                        for p in range(7):
                            ps = ppp.tile([P, MB], mybir.dt.float32,
                                          tag="ps", name="ps")
                            for h in (0, 1):
                                wt = wops[npair * 2 + h][p]
                                for kk in range(KH):
                                    nc.tensor.matmul(
                                        ps[:, h * mbs:h * mbs + mbs],
                                        wt[:, kk * P:(kk + 1) * P],
                                        xops[p][:, kk * mbs:(kk + 1) * mbs],
                                        start=(h == 0 and kk == 0),
                                        stop=(h == 1 and kk == KH - 1))
                            ban.append(ps)
                        # all-add Winograd recombination, <=1 PSUM per op
                        o_t = opo.tile([P, 8 * MBS], mybir.dt.bfloat16,
                                       tag="o", name="o")
                        s = slice(0, 2 * mbs)
                        # bf16 intermediates: ~45% less DVE byte traffic,
                        # keeps the HAM power throttle off the PE
                        # (rel err 9.1e-3 vs 8.3e-3 fp32, budget 2e-2)
                        cp = tpv.tile([P, 2 * MBS], mybir.dt.bfloat16,
                                      tag="cp", name="cp")
                        u2 = tpv.tile([P, 2 * MBS], mybir.dt.bfloat16,
                                      tag="u2", name="u2")
                        u3 = tpv.tile([P, 2 * MBS], mybir.dt.bfloat16,
                                      tag="u3", name="u3")
                        u4 = tpv.tile([P, 2 * MBS], mybir.dt.bfloat16,
                                      tag="u4", name="u4")
                        CPY = mybir.ActivationFunctionType.Copy
                        nc.scalar.activation(cp[:, s], ban[0][:, s], CPY)
                        nc.vector.tensor_add(o_t[:, 0:2 * mbs],
                                             ban[1][:, s], cp[:, s])  # C11
                        nc.vector.tensor_add(u2[:, s], ban[5][:, s],
                                             cp[:, s])
                        nc.vector.tensor_add(u3[:, s], ban[6][:, s],
                                             u2[:, s])
                        nc.vector.tensor_add(u4[:, s], ban[4][:, s],
                                             u2[:, s])
                        nc.vector.tensor_add(o_t[:, 2 * mbs:4 * mbs],
                                             ban[2][:, s], u4[:, s])  # C12
                        nc.vector.tensor_add(o_t[:, 4 * mbs:6 * mbs],
                                             ban[3][:, s], u3[:, s])  # C21
                        nc.vector.tensor_add(o_t[:, 6 * mbs:8 * mbs],
                                             ban[4][:, s], u3[:, s])  # C22
                        if ei == len(m_list) - 1:
                            # tail: last expert's outputs ride all three
                            # queues in halves so the final DMA is small
                            h4 = 4 * mbs
                            t_engs = [nc.sync, nc.scalar]
                            for ti in range(2):
                                t_engs[ti].dma_start(
                                    out=out[:, col0 + ti * h4:
                                            col0 + (ti + 1) * h4],
                                    in_=o_t[:, ti * h4:(ti + 1) * h4])
                        else:
                            nc.gpsimd.dma_start(
                                out=out[:, col0:col0 + 8 * mbs],
                                in_=o_t[:, :8 * mbs])
                        col0 += 8 * mbs
                    b0 += mbs
    nc.compile()
    return nc


def pack_x_strassen(x_padded, order, m_all, K):
    parts = []
    for e in order:
        m = m_all[e]
        mh = (m + 1) // 2
        xp2 = np.zeros((2 * mh, K), dtype=np.float32)
        xp2[:m] = x_padded[e, :m, :]
        b0 = 0
        for mbs in blocks_of_s(mh):
            for r in (0, 1):
                for g in (0, 1):
                    q = xp2[r * mh + b0:r * mh + b0 + mbs,
                            g * (K // 2):(g + 1) * (K // 2)]
                    b = q.T.reshape(KH, P, mbs).transpose(1, 0, 2)
                    row = np.zeros((P, KH * MBS), dtype=np.float32)
                    row[:, :KH * mbs] = b.reshape(P, KH * mbs)
                    parts.append(row)
            b0 += mbs
    return to_bf16(np.concatenate(parts, axis=0))


def pack_w_strassen(stacked_weights, order, c, NC_N, K):
    parts = []
    for e in order:
        blkw = stacked_weights[e, c * NC_N:(c + 1) * NC_N, :]  # [1024, K]
        for nn in range(NH):
            for (h, g) in ((0, 0), (0, 1), (1, 0), (1, 1)):
                q = blkw[h * 512 + nn * P:h * 512 + (nn + 1) * P,
                         g * (K // 2):(g + 1) * (K // 2)]  # [128, 1024]
                # stationary: [p, kk*128 + j] = q[j, kk*128 + p]
                a = q.reshape(P, KH, P).transpose(2, 1, 0)
                parts.append(np.ascontiguousarray(a).reshape(P, KH * P))
    return to_bf16(np.concatenate(parts, axis=0))


def kernel_strassen(x_padded, stacked_weights, m_sizes):
    global LAST_RESULT
    x_padded = np.ascontiguousarray(np.asarray(x_padded, dtype=np.float32))
    stacked_weights = np.ascontiguousarray(
        np.asarray(stacked_weights, dtype=np.float32))
    E, MAX_M, K = x_padded.shape
    N = stacked_weights.shape[1]
    NC_N = N // N_CORES
    m_all = [int(min(max(int(mm), 0), MAX_M))
             for mm in np.asarray(m_sizes).astype(np.int64)]

    out_full = np.zeros((E, MAX_M, N), dtype=np.float32)
    order = [e for e in range(E) if m_all[e] > 0]
    if not order:
        return out_full
    order.sort(key=lambda e: -m_all[e])
    # interleave big/small so small experts' w prefetch hides under the
    # preceding big expert's long PE stretch (small experts are w-bound)
    desc = order
    inter = []
    i, j = 0, len(desc) - 1
    while i <= j:
        inter.append(desc[i])
        if j > i:
            inter.append(desc[j])
        i += 1
        j -= 1
    order = inter
    m_list = [m_all[e] for e in order]

    _install_profile_shim()
    key = ("strassen", tuple(m_list))
    if key not in _NC_CACHE:
        _NC_CACHE[key] = build_nc_strassen(m_list)
    nc = _NC_CACHE[key]

    xsw = pack_x_strassen(x_padded, order, m_all, K)
    in_maps = [{"xsw": xsw,
                "wsw": pack_w_strassen(stacked_weights, order, c, NC_N, K)}
               for c in range(N_CORES)]
    res = run_bass_kernel_spmd(nc, in_maps, list(range(N_CORES)))
    LAST_RESULT = res

    for c in range(N_CORES):
        o = np.asarray(res.results[c]["out"]).astype(np.float32)
        col = 0
        for i, e in enumerate(order):
            m = m_list[i]
            mh = (m + 1) // 2
            b0 = 0
            for mbs in blocks_of_s(mh):
                for npair in range(NH // 2):
                    for (r, h) in ((0, 0), (0, 1), (1, 0), (1, 1)):
                        for hh in (0, 1):
                            nn = npair * 2 + hh
                            t = o[:, col:col + mbs]  # [p, j]
                            r0 = r * mh + b0
                            r1 = min(r0 + mbs, m)
                            if r1 > r0:
                                n0 = c * NC_N + h * 512 + nn * P
                                out_full[e, r0:r1, n0:n0 + P] = \
                                    t[:, :r1 - r0].T
                            col += mbs
                b0 += mbs
    return out_full


def pack_x(x_padded, order, m_all, K):
    """x^T packed per (expert, m-block): [128, KK*mbs] rows, padded to
    the fixed [P, KK*MB] dram row width."""
    KK = K // P
    parts = []
    for e in order:
        m = m_all[e]
        mb0 = 0
        for mbs in blocks_of(m):
            b = x_padded[e, mb0:mb0 + mbs, :].T        # [K, mbs]
            b = b.reshape(KK, P, mbs).transpose(1, 0, 2)  # [P, KK, mbs]
            row = np.zeros((P, KK * MB), dtype=np.float32)
            row[:, :KK * mbs] = b.reshape(P, KK * mbs)
            parts.append(row)
            mb0 += mbs
    return to_bf16(np.concatenate(parts, axis=0))


def pack_w(stacked_weights, order, c, NC_N, K):
    """Weights per (expert, nn): [128, KK*P] with
    [p, kk*P+j] = w[e, c*NC_N + nn*P + j, kk*P + p]."""
    KK = K // P
    NN = NC_N // P
    parts = []
    for e in order:
        blk = stacked_weights[e, c * NC_N:(c + 1) * NC_N, :]  # [NC_N, K]
        a = blk.reshape(NN, P, KK, P).transpose(0, 3, 2, 1)   # [nn, p, kk, j]
        parts.append(np.ascontiguousarray(a).reshape(NN * P, KK * P))
    return to_bf16(np.concatenate(parts, axis=0))


def kernel(x_padded, stacked_weights, m_sizes):
    import os
    if os.environ.get("BASS_IMPL", "strassen") == "strassen":
        return kernel_strassen(x_padded, stacked_weights, m_sizes)
    return kernel_base(x_padded, stacked_weights, m_sizes)


def kernel_base(x_padded, stacked_weights, m_sizes):
    global LAST_RESULT
    x_padded = np.ascontiguousarray(np.asarray(x_padded, dtype=np.float32))
    stacked_weights = np.ascontiguousarray(
        np.asarray(stacked_weights, dtype=np.float32))
    E, MAX_M, K = x_padded.shape
    N = stacked_weights.shape[1]
    NC_N = N // N_CORES
    NN = NC_N // P
    m_all = [int(min(max(int(mm), 0), MAX_M))
             for mm in np.asarray(m_sizes).astype(np.int64)]

    out_full = np.zeros((E, MAX_M, N), dtype=np.float32)
    order = [e for e in range(E) if m_all[e] > 0]
    if not order:
        return out_full
    # descending size: the big first expert hides its own weight load,
    # and the small trailing experts ride 3-expert weight prefetch
    order.sort(key=lambda e: -m_all[e])
    m_list = [m_all[e] for e in order]
    SM = sum(m_list)

    _install_profile_shim()
    nc = get_nc(m_list, K, NC_N)

    xsw = pack_x(x_padded, order, m_all, K)
    in_maps = [{"xsw": xsw,
                "wsw": pack_w(stacked_weights, order, c, NC_N, K)}
               for c in range(N_CORES)]

    res = run_bass_kernel_spmd(nc, in_maps, list(range(N_CORES)))
    LAST_RESULT = res

    for c in range(N_CORES):
        o = np.asarray(res.results[c]["out"]).astype(np.float32)  # [P, NN*SM]
        outT = o.reshape(P, NN, SM).transpose(1, 0, 2).reshape(NC_N, SM)
        col = 0
        for i, e in enumerate(order):
            m = m_list[i]
            out_full[e, :m, c * NC_N:(c + 1) * NC_N] = outT[:, col:col + m].T
            col += m
    return out_full

